# revision 1
# baseline (speedup 1.0000x reference)
"""ContxE-style temporal KG embedding scoring kernel for Trainium2 (Bass/Tile).

Contract: kernel(**inputs) takes FULL unsharded numpy inputs and returns the
FULL [B] float32 output. Internally shards the batch across 8 NeuronCores
(data-parallel, tables replicated) and runs a Bass/Tile kernel via
run_bass_kernel_spmd.

Math (per batch element b, window W=5, D=512):
  idx[b,w] = d[b]-(4-w), clamped: negatives -> 365
  c/s[b,w,:] = cos/sin(time_table[idx[b,w]])
  h_real = hr*c - hi*s ; h_img = hr*s + hi*c   (same for t)
  4 attention softmaxes over W of <r, rotated>, then weighted sums,
  out = sum|y_r + rr - z_r| + sum|y_i + ri + z_i|

Device-side strategy (per core, B_loc=2048 = 16 tiles of 128):
  - ONE indirect-DMA gather per embedding pair (tables concatenated host-side)
  - cos|sin rows come from a host-precomputed bf16 table with 4 prefix rows
    equal to row 365, so the W-window gather is ONE contiguous 10KB read per
    batch element (clamp semantics fall out of the prefix rows)
  - logits via fused tensor_tensor_reduce over [c|s]-interleaved pairs
  - attention-weighted sums via scalar_tensor_tensor accumulation chains
"""

import sys

if "/opt/trn_rl_repo" not in sys.path:
    sys.path.insert(0, "/opt/trn_rl_repo")

import numpy as np
import ml_dtypes

import concourse.bass as bass
import concourse.bacc as bacc
import concourse.tile as tile
from concourse import mybir
from concourse.bass_utils import run_bass_kernel_spmd

N_CORES = 8
B = 16384
BL = B // N_CORES          # 2048 per core
P = 128
T = BL // P                # 16 tiles per core
D = 512
W = 5
N_ENTITY = 100000
N_RELATION = 256
N_DAYROWS = 367            # time_table rows
PAD_DAY = 365              # negatives clamp to this row

F32 = mybir.dt.float32
BF16 = mybir.dt.bfloat16
I32 = mybir.dt.int32

AF = mybir.ActivationFunctionType
OP = mybir.AluOpType


from concourse._compat import with_exitstack


@with_exitstack
def _emit(ctx, tc, outs, ins):
    """Emit the per-core program. outs/ins are dicts of DRAM APs."""
    nc = tc.nc
    embE2 = ins["embE2"]      # [2*N_ENTITY, D] f32   (real rows then img rows)
    embR2 = ins["embR2"]      # [2*N_RELATION, D] f32
    cs_ext = ins["cs_ext"]    # [370, 2*D] bf16 ([cos|sin], 4 prefix rows = row 365)
    ht_idx = ins["ht_idx"]    # [P, T*4] i32  (h, h+NE, t, t+NE per tile col)
    r_idx = ins["r_idx"]      # [P, T*2] i32
    d_idx = ins["d_idx"]      # [P, T]   i32
    out = outs["out"]         # [P, T] f32

    singles = ctx.enter_context(tc.tile_pool(name="singles", bufs=1))
    gpool = ctx.enter_context(tc.tile_pool(name="g", bufs=3))
    upool = ctx.enter_context(tc.tile_pool(name="u", bufs=2))
    wpool = ctx.enter_context(tc.tile_pool(name="w", bufs=2))
    spool = ctx.enter_context(tc.tile_pool(name="s", bufs=2))

    # --- load index tiles + output accumulator (resident) ---
    sb_ht = singles.tile([P, T * 4], I32)
    sb_r = singles.tile([P, T * 2], I32)
    sb_d = singles.tile([P, T], I32)
    out_acc = singles.tile([P, T], F32)
    nc.sync.dma_start(sb_ht[:], ht_idx[:])
    nc.sync.dma_start(sb_r[:], r_idx[:])
    nc.sync.dma_start(sb_d[:], d_idx[:])

    for t in range(T):
        # ---- gathers ----
        g = gpool.tile([P, 4 * D], BF16, tag="g")      # hr|hi|tr|ti
        rg = gpool.tile([P, 2 * D], BF16, tag="rg")    # rr|ri
        cs = gpool.tile([P, W * 2 * D], BF16, tag="cs")  # per w: [c_w | s_w]

        for c in range(4):
            nc.gpsimd.indirect_dma_start(
                out=g[:, c * D:(c + 1) * D],
                out_offset=None,
                in_=embE2[:],
                in_offset=bass.IndirectOffsetOnAxis(
                    ap=sb_ht[:, t * 4 + c: t * 4 + c + 1], axis=0
                ),
            )
        for c in range(2):
            nc.gpsimd.indirect_dma_start(
                out=rg[:, c * D:(c + 1) * D],
                out_offset=None,
                in_=embR2[:],
                in_offset=bass.IndirectOffsetOnAxis(
                    ap=sb_r[:, t * 2 + c: t * 2 + c + 1], axis=0
                ),
            )
        nc.gpsimd.indirect_dma_start(
            out=cs[:],
            out_offset=None,
            in_=cs_ext[:],
            in_offset=bass.IndirectOffsetOnAxis(ap=sb_d[:, t: t + 1], axis=0),
        )

        hr = g[:, 0 * D:1 * D]
        hi = g[:, 1 * D:2 * D]
        tr = g[:, 2 * D:3 * D]
        ti = g[:, 3 * D:4 * D]
        rr = rg[:, 0 * D:1 * D]
        ri = rg[:, 1 * D:2 * D]

        # ---- u-pairs: coefficient of c | coefficient of s for each logit type
        # type 0 (real,h): [ rr*hr | -rr*hi ]
        # type 1 (img ,h): [ ri*hi |  ri*hr ]
        # type 2 (real,t): [ rr*tr | -rr*ti ]
        # type 3 (img ,t): [ ri*ti |  ri*tr ]
        U = upool.tile([P, 4, 2 * D], BF16, tag="U")
        nrr = spool.tile([P, D], BF16, tag="nrr")
        nc.vector.tensor_scalar(out=nrr[:], in0=rr, scalar1=-1.0, scalar2=None,
                                op0=OP.mult)
        nc.vector.tensor_tensor(out=U[:, 0, 0:D], in0=rr, in1=hr, op=OP.mult)
        nc.vector.tensor_tensor(out=U[:, 0, D:2 * D], in0=nrr[:], in1=hi,
                                op=OP.mult)
        nc.vector.tensor_tensor(out=U[:, 1, 0:D], in0=ri, in1=hi, op=OP.mult)
        nc.vector.tensor_tensor(out=U[:, 1, D:2 * D], in0=ri, in1=hr, op=OP.mult)
        nc.vector.tensor_tensor(out=U[:, 2, 0:D], in0=rr, in1=tr, op=OP.mult)
        nc.vector.tensor_tensor(out=U[:, 2, D:2 * D], in0=nrr[:], in1=ti,
                                op=OP.mult)
        nc.vector.tensor_tensor(out=U[:, 3, 0:D], in0=ri, in1=ti, op=OP.mult)
        nc.vector.tensor_tensor(out=U[:, 3, D:2 * D], in0=ri, in1=tr, op=OP.mult)

        # ---- logits: L[b, ty, w] = sum(U[ty] * cs[w]) ----
        # DVE: one broadcast TT per type over all 5 windows;
        # ACT: per-(ty,w) Copy with accum_out does the reduction.
        L = spool.tile([P, 4 * W], F32, tag="L")
        dummy = spool.tile([P, 2 * D], BF16, tag="dummy")
        csv = cs.rearrange("p (w e) -> p w e", w=W)
        # types 0,1: fused STT dot on DVE; types 2,3: DVE broadcast-mult
        # + ACT accum reduce (balances DVE vs ACT)
        for ty in range(2):
            for w in range(W):
                nc.vector.scalar_tensor_tensor(
                    out=dummy[:], in0=U[:, ty, :], scalar=1.0,
                    in1=csv[:, w, :], op0=OP.mult, op1=OP.mult,
                    accum_out=L[:, ty * W + w: ty * W + w + 1])
        for ty in range(2, 4):
            prod5 = wpool.tile([P, W, 2 * D], BF16, tag="prod5")
            ub = U[:, ty: ty + 1, :].to_broadcast([P, W, 2 * D])
            nc.vector.tensor_tensor(out=prod5[:], in0=ub, in1=csv, op=OP.mult)
            for w in range(W):
                nc.scalar.activation(
                    dummy[:], prod5[:, w, :], AF.Copy,
                    accum_out=L[:, ty * W + w: ty * W + w + 1])

        # ---- softmax over w (logits are O(1); skip max-subtraction) ----
        Ex = spool.tile([P, 4 * W], F32, tag="Ex")
        Sm = spool.tile([P, 4], F32, tag="Sm")
        Rc = spool.tile([P, 4], F32, tag="Rc")
        Al = spool.tile([P, 4 * W], F32, tag="Al")
        nc.scalar.activation(Ex[:], L[:], AF.Exp)
        nc.vector.tensor_reduce(
            out=Sm[:], in_=Ex.rearrange("p (t w) -> p t w", w=W),
            axis=mybir.AxisListType.X, op=OP.add)
        nc.vector.reciprocal(Rc[:], Sm[:])
        for ty in range(4):
            nc.vector.tensor_scalar(
                out=Al[:, ty * W:(ty + 1) * W],
                in0=Ex[:, ty * W:(ty + 1) * W],
                scalar1=Rc[:, ty: ty + 1],
                scalar2=None,
                op0=OP.mult,
            )

        # ---- attention-weighted sums: CSS[ty] = sum_w alpha[ty,w]*cs[w] ----
        # ACT: 5 scaled copies (scale = alpha per partition); DVE: tree-add.
        CSS = wpool.tile([P, 4, 2 * D], BF16, tag="CSS")
        for ty in range(4):
            ap5 = wpool.tile([P, W, 2 * D], BF16, tag="ap5")
            for w in range(W):
                # split scaled copies between ACT and DVE-TS; types 0-1
                # lean more on ACT (their logit dots run on DVE)
                if w < (4 if ty < 2 else 3):
                    nc.scalar.activation(
                        ap5[:, w, :], csv[:, w, :], AF.Copy,
                        scale=Al[:, ty * W + w: ty * W + w + 1])
                else:
                    nc.vector.tensor_scalar(
                        out=ap5[:, w, :], in0=csv[:, w, :],
                        scalar1=Al[:, ty * W + w: ty * W + w + 1],
                        scalar2=None, op0=OP.mult)
            t12 = spool.tile([P, 2, 2 * D], BF16, tag="t12")
            # one wide add: (p0+p2 | p1+p3), then fold halves, then +p4
            nc.vector.tensor_tensor(
                out=t12[:], in0=ap5[:, 0:2, :], in1=ap5[:, 2:4, :], op=OP.add)
            nc.vector.tensor_tensor(out=t12[:, 0, :], in0=t12[:, 0, :],
                                    in1=t12[:, 1, :], op=OP.add)
            nc.vector.tensor_tensor(out=CSS[:, ty, :], in0=t12[:, 0, :],
                                    in1=ap5[:, 4, :], op=OP.add)

        # ---- recombine: y/z vectors [P, D] ----
        # y_r = hr*Cc0 - hi*Cs0 ; y_i = hr*Cs1 + hi*Cc1
        # z_r = tr*Cc2 - ti*Cs2 ; z_i = tr*Cs3 + ti*Cc3
        p1 = spool.tile([P, D], BF16, tag="p1")
        p2 = spool.tile([P, D], BF16, tag="p2")
        yz = wpool.tile([P, 4, D], BF16, tag="yz")
        specs = [
            (hr, CSS[:, 0, 0:D], hi, CSS[:, 0, D:2 * D], OP.subtract),  # y_r
            (hr, CSS[:, 1, D:2 * D], hi, CSS[:, 1, 0:D], OP.add),       # y_i
            (tr, CSS[:, 2, 0:D], ti, CSS[:, 2, D:2 * D], OP.subtract),  # z_r
            (tr, CSS[:, 3, D:2 * D], ti, CSS[:, 3, 0:D], OP.add),       # z_i
        ]
        for k, (a0, b0, a1, b1, opk) in enumerate(specs):
            # y_i/z_r/z_i products+combine go to GpSimd to offload DVE
            eng = nc.vector if k < 1 else nc.gpsimd
            pa = p1 if k < 1 else spool.tile([P, D], BF16, tag=f"gp{k}a")
            pb = p2 if k < 1 else spool.tile([P, D], BF16, tag=f"gp{k}b")
            eng.tensor_tensor(out=pa[:], in0=a0, in1=b0, op=OP.mult)
            eng.tensor_tensor(out=pb[:], in0=a1, in1=b1, op=OP.mult)
            eng.tensor_tensor(out=yz[:, k, :], in0=pa[:], in1=pb[:], op=opk)

        # ---- final: out = sum|y_r + rr - z_r| + sum|y_i + ri + z_i| ----
        f1 = spool.tile([P, D], BF16, tag="f1")
        f2 = spool.tile([P, D], BF16, tag="f2")
        o_r = spool.tile([P, 1], F32, tag="o_r")
        o_i = spool.tile([P, 1], F32, tag="o_i")
        nc.vector.tensor_tensor(out=f1[:], in0=yz[:, 0, :], in1=rr, op=OP.add)
        nc.vector.tensor_tensor(out=f2[:], in0=f1[:], in1=yz[:, 2, :], op=OP.subtract)
        nc.vector.tensor_reduce(
            out=o_r[:], in_=f2[:], axis=mybir.AxisListType.X, op=OP.add,
            apply_absolute_value=True)
        nc.vector.tensor_tensor(out=f1[:], in0=yz[:, 1, :], in1=ri, op=OP.add)
        nc.vector.tensor_tensor(out=f2[:], in0=f1[:], in1=yz[:, 3, :], op=OP.add)
        nc.vector.tensor_reduce(
            out=o_i[:], in_=f2[:], axis=mybir.AxisListType.X, op=OP.add,
            apply_absolute_value=True)
        nc.vector.tensor_tensor(
            out=out_acc[:, t: t + 1], in0=o_r[:], in1=o_i[:], op=OP.add)

    nc.sync.dma_start(out[:], out_acc[:])


def _host_prep(h_i, t_i, r_i, d_i, emb_E_real, emb_E_img, emb_R_real,
               emb_R_img, time_table):
    """Host-side layout prep (cheap index/table manipulation only)."""
    embE2 = np.ascontiguousarray(
        np.concatenate([emb_E_real, emb_E_img], axis=0), dtype=np.float32)
    embR2 = np.ascontiguousarray(
        np.concatenate([emb_R_real, emb_R_img], axis=0), dtype=np.float32)
    tt = np.asarray(time_table, dtype=np.float32)
    cs = np.concatenate([np.cos(tt), np.sin(tt)], axis=1)  # [367, 1024] f32
    # 4 prefix rows equal to row PAD_DAY implement the negative-index clamp;
    # row d of the original table sits at ext row d+4, so a window gather for
    # batch element b is rows d[b] .. d[b]+4 of cs_ext == one contiguous read.
    cs_ext = np.concatenate(
        [np.broadcast_to(cs[PAD_DAY], (4, 2 * D)), cs[:366]], axis=0)
    cs_ext = np.ascontiguousarray(cs_ext, dtype=ml_dtypes.bfloat16)  # [370, 1024]

    ht = np.stack(
        [h_i, h_i + N_ENTITY, t_i, t_i + N_ENTITY], axis=1).astype(np.int32)
    rx = np.stack([r_i, r_i + N_RELATION], axis=1).astype(np.int32)
    dx = d_i.astype(np.int32).reshape(B, 1)

    def tileize(a):
        # [BL, C] -> [P, T*C] with element [p, t*C+c] = a[t*P+p, c]
        C = a.shape[1]
        return np.ascontiguousarray(
            a.reshape(T, P, C).transpose(1, 0, 2).reshape(P, T * C))

    in_maps = []
    for core in range(N_CORES):
        sl = slice(core * BL, (core + 1) * BL)
        in_maps.append(dict(
            embE2=embE2,
            embR2=embR2,
            cs_ext=cs_ext,
            ht_idx=tileize(ht[sl]),
            r_idx=tileize(rx[sl]),
            d_idx=tileize(dx[sl]),
        ))
    return in_maps


def build_nc():
    nc = bacc.Bacc(
        "TRN2",
        target_bir_lowering=False,
        debug=False,
        enable_asserts=False,
        num_devices=N_CORES,
    )
    ins = dict(
        embE2=nc.dram_tensor("embE2", [2 * N_ENTITY, D], F32,
                             kind="ExternalInput").ap(),
        embR2=nc.dram_tensor("embR2", [2 * N_RELATION, D], F32,
                             kind="ExternalInput").ap(),
        cs_ext=nc.dram_tensor("cs_ext", [370, 2 * D], BF16,
                              kind="ExternalInput").ap(),
        ht_idx=nc.dram_tensor("ht_idx", [P, T * 4], I32,
                              kind="ExternalInput").ap(),
        r_idx=nc.dram_tensor("r_idx", [P, T * 2], I32,
                             kind="ExternalInput").ap(),
        d_idx=nc.dram_tensor("d_idx", [P, T], I32,
                             kind="ExternalInput").ap(),
    )
    outs = dict(
        out=nc.dram_tensor("out", [P, T], F32, kind="ExternalOutput").ap(),
    )
    with tile.TileContext(nc) as tc:
        _emit(tc, outs, ins)
    nc.compile()
    return nc


_NC_CACHE = {}


def kernel(h_i, t_i, r_i, d_i, emb_E_real, emb_E_img, emb_R_real, emb_R_img,
           time_table, _want_results=False, _trace=False):
    in_maps = _host_prep(h_i, t_i, r_i, d_i, emb_E_real, emb_E_img,
                         emb_R_real, emb_R_img, time_table)
    if "nc" not in _NC_CACHE:
        _NC_CACHE["nc"] = build_nc()
    nc = _NC_CACHE["nc"]
    res = run_bass_kernel_spmd(
        nc, in_maps, core_ids=list(range(N_CORES)), trace=_trace)
    out = np.empty((B,), np.float32)
    for core in range(N_CORES):
        o = res.results[core]["out"]  # [P, T]
        out[core * BL:(core + 1) * BL] = np.asarray(o).T.reshape(BL)
    if _want_results:
        return out, res
    return out



# revision 5
# speedup vs baseline: 1.8653x; 1.8653x over previous
"""ContxE-style temporal KG embedding scoring kernel for Trainium2 (Bass/Tile).

Contract: kernel(**inputs) takes FULL unsharded numpy inputs and returns the
FULL [B] float32 output. Internally shards the batch across 8 NeuronCores
(data-parallel, tables replicated) and runs a Bass/Tile kernel via
run_bass_kernel_spmd.

Math (per batch element b, window W=5, D=512):
  idx[b,w] = d[b]-(4-w), clamped: negatives -> 365
  c/s[b,w,:] = cos/sin(time_table[idx[b,w]])
  h_real = hr*c - hi*s ; h_img = hr*s + hi*c   (same for t)
  4 attention softmaxes over W of <r, rotated>, then weighted sums,
  out = sum|y_r + rr - z_r| + sum|y_i + ri + z_i|

Device-side strategy (per core, B_loc=2048, 4 blocks of 512):
  The two per-element contractions run on the TensorEngine against the
  (small, replicated) extended cos|sin table rather than on DVE:
    phase A:  V[i,b] = <U_ty[b,:], cs_ext[i,:]> for ALL 384 padded table
              rows i as a matmul (stationary = transposed cs table,
              moving = U^T).  The 5 window logits are V[day..day+4, b].
    masked exp:  E = exp(V) * mask  (mask[i,b] = day_b <= i <= day_b+4,
              host-precomputed) gives softmax numerators already in
              [i, b] layout -- no gather/scatter.
    phase B:  CSS[b,:] = E.T @ cs_ext (+ ones column for the softmax
              denominator D), landing back in [b, d] layout; the 1/D
              normalization is folded into the ACT PSUM->SBUF copy as a
              per-partition scale.
  U^T ([d', b] layout) is built from embedding factors transposed via a
  DRAM round-trip with xbar DMA-transpose. Embedding gathers use paired
  [real|img] bf16 rows (one 2KB indirect-DMA row per entity).
"""

import sys

if "/opt/trn_rl_repo" not in sys.path:
    sys.path.insert(0, "/opt/trn_rl_repo")

import numpy as np
import ml_dtypes

import concourse.bass as bass
import concourse.bacc as bacc
import concourse.tile as tile
from concourse import mybir
from concourse.bass_utils import run_bass_kernel_spmd

N_CORES = 8
B = 16384
BL = B // N_CORES          # 2048 per core
P = 128
T = BL // P                # 16 tiles of 128 per core
D = 512
DD = 2 * D                 # 1024 (cos|sin pair width)
W = 5
N_ENTITY = 100000
N_RELATION = 256
PAD_DAY = 365
NI = 384                   # padded extended-table rows (370 used)
IC = NI // P               # 3 i-chunks
JC = DD // P               # 8 d'-chunks
BLK = 512                  # batch block
NBLK = BL // BLK           # 4
SUB = BLK // P             # 4 sub-tiles of 128 per block

F32 = mybir.dt.float32
BF16 = mybir.dt.bfloat16
I32 = mybir.dt.int32

AF = mybir.ActivationFunctionType
OP = mybir.AluOpType


from concourse._compat import with_exitstack


@with_exitstack
def _emit(ctx, tc, outs, ins):
    """Emit the per-core program. outs/ins are dicts of DRAM APs."""
    nc = tc.nc
    embEp = ins["embEp"]      # [N_ENTITY, 1024] bf16  ([real|img] paired rows)
    embRp = ins["embRp"]      # [N_RELATION, 1024] bf16
    csF_d = ins["csF"]        # [128, IC*DD]  bf16  forward ext table, chunked
    csT_d = ins["csT"]        # [128, JC*NI]  bf16  transposed ext table
    mask_d = ins["maskT"]     # [128, IC*BL]  bf16  window mask [i, b]
    ht_idx = ins["ht_idx"]    # [P, T*2] i32  (h, t per tile col)
    r_idx = ins["r_idx"]      # [P, T]   i32
    out = outs["out"]         # [P, T] f32

    singles = ctx.enter_context(tc.tile_pool(name="singles", bufs=1))
    gpool = ctx.enter_context(tc.tile_pool(name="g", bufs=2))
    tpool = ctx.enter_context(tc.tile_pool(name="t", bufs=1))
    upool = ctx.enter_context(tc.tile_pool(name="u", bufs=1))
    epool = ctx.enter_context(tc.tile_pool(name="e", bufs=2))
    apool = ctx.enter_context(tc.tile_pool(name="a", bufs=2))
    wpool = ctx.enter_context(tc.tile_pool(name="w", bufs=1))
    vpsum = ctx.enter_context(tc.tile_pool(name="vps", bufs=1, space="PSUM"))
    cpsum = ctx.enter_context(tc.tile_pool(name="cps", bufs=2, space="PSUM"))
    dpsum = ctx.enter_context(tc.tile_pool(name="dps", bufs=2, space="PSUM"))
    dram = ctx.enter_context(tc.tile_pool(name="dram", bufs=2, space="DRAM"))

    # --- resident tables / indices ---
    csF = singles.tile([P, IC, DD], BF16)    # csF[p,k,:] = cs_pad[k*128+p,:]
    csT = singles.tile([P, JC, NI], BF16)    # csT[p,j,i] = cs_pad[i,j*128+p]
    mask = singles.tile([P, IC, BL], BF16)   # mask[p,k,b]
    sb_ht = singles.tile([P, T * 2], I32)
    sb_r = singles.tile([P, T], I32)
    ones = singles.tile([P, 1], BF16)
    out_acc = singles.tile([P, T], F32)
    nc.sync.dma_start(csF[:], csF_d.rearrange("p (k n) -> p k n", k=IC))
    nc.sync.dma_start(csT[:], csT_d.rearrange("p (j n) -> p j n", j=JC))
    nc.sync.dma_start(mask[:], mask_d.rearrange("p (k n) -> p k n", k=IC))
    nc.sync.dma_start(sb_ht[:], ht_idx[:])
    nc.sync.dma_start(sb_r[:], r_idx[:])
    nc.vector.memset(ones[:], 1.0)

    for blk in range(NBLK):
        # ---- gathers: paired [real|img] rows ----
        hp = gpool.tile([P, SUB, DD], BF16, tag="hp")
        tp = gpool.tile([P, SUB, DD], BF16, tag="tp")
        rp = gpool.tile([P, SUB, DD], BF16, tag="rp")
        for st in range(SUB):
            t_g = blk * SUB + st
            nc.gpsimd.indirect_dma_start(
                out=hp[:, st, :], out_offset=None, in_=embEp[:],
                in_offset=bass.IndirectOffsetOnAxis(
                    ap=sb_ht[:, 2 * t_g: 2 * t_g + 1], axis=0))
            nc.gpsimd.indirect_dma_start(
                out=tp[:, st, :], out_offset=None, in_=embEp[:],
                in_offset=bass.IndirectOffsetOnAxis(
                    ap=sb_ht[:, 2 * t_g + 1: 2 * t_g + 2], axis=0))
            nc.gpsimd.indirect_dma_start(
                out=rp[:, st, :], out_offset=None, in_=embRp[:],
                in_offset=bass.IndirectOffsetOnAxis(
                    ap=sb_r[:, t_g: t_g + 1], axis=0))

        # ---- transpose factors via DRAM round-trip + xbar transpose ----
        hs = dram.tile([BLK, DD], BF16, tag="hs")
        ts_ = dram.tile([BLK, DD], BF16, tag="ts")
        rs = dram.tile([BLK, DD], BF16, tag="rs")
        nc.sync.dma_start(hs.rearrange("(st p) d -> p st d", p=P), hp[:])
        nc.sync.dma_start(ts_.rearrange("(st p) d -> p st d", p=P), tp[:])
        nc.sync.dma_start(rs.rearrange("(st p) d -> p st d", p=P), rp[:])
        hT = tpool.tile([P, JC, BLK], BF16, tag="hT")
        tT = tpool.tile([P, JC, BLK], BF16, tag="tT")
        rT = tpool.tile([P, JC, BLK], BF16, tag="rT")
        for j in range(JC):
            nc.sync.dma_start_transpose(hT[:, j, :], hs[:, j * P:(j + 1) * P])
            nc.sync.dma_start_transpose(tT[:, j, :], ts_[:, j * P:(j + 1) * P])
            nc.sync.dma_start_transpose(rT[:, j, :], rs[:, j * P:(j + 1) * P])

        # ---- U^T build: U[p, ty, j, b] = U_ty[d'=j*128+p, b] ----
        # ty0 = [rr*hr | -rr*hi], ty1 = [ri*hi | ri*hr],
        # ty2 = [rr*tr | -rr*ti], ty3 = [ri*ti | ri*tr]
        U = upool.tile([P, 4, JC, BLK], BF16, tag="U")
        nr = wpool.tile([P, JC // 2, BLK], BF16, tag="nr")
        nc.vector.tensor_scalar(out=nr[:], in0=rT[:, 0:4, :], scalar1=-1.0,
                                scalar2=None, op0=OP.mult)
        nc.vector.tensor_tensor(out=U[:, 0, 0:4, :], in0=rT[:, 0:4, :],
                                in1=hT[:, 0:4, :], op=OP.mult)
        nc.vector.tensor_tensor(out=U[:, 0, 4:8, :], in0=nr[:],
                                in1=hT[:, 4:8, :], op=OP.mult)
        nc.vector.tensor_tensor(out=U[:, 1, 0:4, :], in0=rT[:, 4:8, :],
                                in1=hT[:, 4:8, :], op=OP.mult)
        nc.vector.tensor_tensor(out=U[:, 1, 4:8, :], in0=rT[:, 4:8, :],
                                in1=hT[:, 0:4, :], op=OP.mult)
        nc.vector.tensor_tensor(out=U[:, 2, 0:4, :], in0=rT[:, 0:4, :],
                                in1=tT[:, 0:4, :], op=OP.mult)
        nc.vector.tensor_tensor(out=U[:, 2, 4:8, :], in0=nr[:],
                                in1=tT[:, 4:8, :], op=OP.mult)
        nc.vector.tensor_tensor(out=U[:, 3, 0:4, :], in0=rT[:, 4:8, :],
                                in1=tT[:, 4:8, :], op=OP.mult)
        nc.vector.tensor_tensor(out=U[:, 3, 4:8, :], in0=rT[:, 4:8, :],
                                in1=tT[:, 0:4, :], op=OP.mult)

        # ---- phase A: V[i,b] per i-chunk, then E = exp(V)*mask ----
        E = epool.tile([P, 4, IC, BLK], BF16, tag="E")
        for k in range(IC):
            for tp2 in range(2):           # ty pairs share stationary loads
                vts = vpsum.tile([P, 2, BLK], F32, tag="vts")
                for j in range(JC):
                    lhsT = csT[:, j, k * P:(k + 1) * P]
                    for tyh in range(2):
                        ty = tp2 * 2 + tyh
                        nc.tensor.matmul(
                            vts[:, tyh, :], lhsT=lhsT, rhs=U[:, ty, j, :],
                            start=(j == 0), stop=(j == JC - 1))
                for tyh in range(2):
                    ty = tp2 * 2 + tyh
                    nc.scalar.activation(E[:, ty, k, :], vts[:, tyh, :], AF.Exp)
                    nc.vector.tensor_tensor(
                        out=E[:, ty, k, :], in0=E[:, ty, k, :],
                        in1=mask[:, k, blk * BLK:(blk + 1) * BLK], op=OP.mult)

        # ---- phase B + C per 128-row sub-tile ----
        for s in range(SUB):
            bs = slice(s * P, (s + 1) * P)
            dps = dpsum.tile([P, 4], F32, tag="dps")
            A = apool.tile([P, 4, DD], BF16, tag="A")
            rd = wpool.tile([P, 4], F32, tag="rd")
            css = []
            for ty in range(4):
                cps = cpsum.tile([P, DD], F32, tag="cps")
                swap = ty in (1, 3)   # store CSS as [As|Ac] for img types
                for k in range(IC):
                    st_, sp_ = (k == 0), (k == IC - 1)
                    lhsT = E[:, ty, k, bs]
                    lo = csF[:, k, D:DD] if swap else csF[:, k, 0:D]
                    hi = csF[:, k, 0:D] if swap else csF[:, k, D:DD]
                    nc.tensor.matmul(cps[:, 0:D], lhsT=lhsT, rhs=lo,
                                     start=st_, stop=sp_)
                    nc.tensor.matmul(cps[:, D:DD], lhsT=lhsT, rhs=hi,
                                     start=st_, stop=sp_)
                    nc.tensor.matmul(dps[:, ty:ty + 1], lhsT=lhsT,
                                     rhs=ones[:, 0:1], start=st_, stop=sp_)
                css.append(cps)
            nc.vector.reciprocal(rd[:], dps[:])
            for ty in range(4):
                nc.scalar.activation(A[:, ty, :], css[ty][:], AF.Copy,
                                     scale=rd[:, ty:ty + 1])

            # recombine in [b, d] layout
            # G = [hr*A0c | hi*A0s | hr*A1s | hi*A1c], H same with t/A2/A3
            G = wpool.tile([P, 2, DD], BF16, tag="G")
            H = wpool.tile([P, 2, DD], BF16, tag="H")
            nc.vector.tensor_tensor(
                out=G[:], in0=hp[:, s, None, :].broadcast_to([P, 2, DD]),
                in1=A[:, 0:2, :], op=OP.mult)
            nc.vector.tensor_tensor(
                out=H[:], in0=tp[:, s, None, :].broadcast_to([P, 2, DD]),
                in1=A[:, 2:4, :], op=OP.mult)
            yr = wpool.tile([P, D], BF16, tag="yr")
            yi = wpool.tile([P, D], BF16, tag="yi")
            zr = wpool.tile([P, D], BF16, tag="zr")
            zi = wpool.tile([P, D], BF16, tag="zi")
            nc.vector.tensor_tensor(out=yr[:], in0=G[:, 0, 0:D],
                                    in1=G[:, 0, D:DD], op=OP.subtract)
            nc.vector.tensor_tensor(out=yi[:], in0=G[:, 1, 0:D],
                                    in1=G[:, 1, D:DD], op=OP.add)
            nc.vector.tensor_tensor(out=zr[:], in0=H[:, 0, 0:D],
                                    in1=H[:, 0, D:DD], op=OP.subtract)
            nc.vector.tensor_tensor(out=zi[:], in0=H[:, 1, 0:D],
                                    in1=H[:, 1, D:DD], op=OP.add)
            f1 = wpool.tile([P, D], BF16, tag="f1")
            f2 = wpool.tile([P, D], BF16, tag="f2")
            o_r = wpool.tile([P, 1], F32, tag="o_r")
            o_i = wpool.tile([P, 1], F32, tag="o_i")
            dm = wpool.tile([P, D], BF16, tag="dm")
            # f1 = yr - zr + rr ; f2 = yi + zi + ri
            nc.vector.scalar_tensor_tensor(
                out=f1[:], in0=zr[:], scalar=-1.0, in1=yr[:],
                op0=OP.mult, op1=OP.add)
            nc.vector.tensor_tensor(out=f1[:], in0=f1[:], in1=rp[:, s, 0:D],
                                    op=OP.add)
            nc.vector.tensor_tensor(out=f2[:], in0=yi[:], in1=zi[:], op=OP.add)
            nc.vector.tensor_tensor(out=f2[:], in0=f2[:], in1=rp[:, s, D:DD],
                                    op=OP.add)
            nc.scalar.activation(dm[:], f1[:], AF.Abs, accum_out=o_r[:])
            nc.scalar.activation(dm[:], f2[:], AF.Abs, accum_out=o_i[:])
            nc.vector.tensor_tensor(
                out=out_acc[:, blk * SUB + s: blk * SUB + s + 1],
                in0=o_r[:], in1=o_i[:], op=OP.add)

    nc.sync.dma_start(out[:], out_acc[:])


def _host_prep(h_i, t_i, r_i, d_i, emb_E_real, emb_E_img, emb_R_real,
               emb_R_img, time_table):
    """Host-side layout prep (table packing / index manipulation only)."""
    embEp = np.ascontiguousarray(
        np.concatenate([emb_E_real, emb_E_img], axis=1)).astype(
            ml_dtypes.bfloat16)                       # [N_ENTITY, 1024]
    embRp = np.ascontiguousarray(
        np.concatenate([emb_R_real, emb_R_img], axis=1)).astype(
            ml_dtypes.bfloat16)                       # [N_RELATION, 1024]

    tt = np.asarray(time_table, dtype=np.float32)
    cs = np.concatenate([np.cos(tt), np.sin(tt)], axis=1)  # [367, 1024]
    cs_pad = np.zeros((NI, DD), np.float32)
    cs_pad[0:4] = cs[PAD_DAY]          # prefix rows implement neg-idx clamp
    cs_pad[4:370] = cs[0:366]
    # forward table, chunked for SBUF [128, IC, DD]
    csF = np.ascontiguousarray(
        cs_pad.reshape(IC, P, DD).transpose(1, 0, 2).reshape(P, IC * DD)
    ).astype(ml_dtypes.bfloat16)
    # transposed table for SBUF [128, JC, NI]: csT[p, j, i] = cs_pad[i, j*128+p]
    csT = np.ascontiguousarray(
        cs_pad.T.reshape(JC, P, NI).transpose(1, 0, 2).reshape(P, JC * NI)
    ).astype(ml_dtypes.bfloat16)

    d = np.asarray(d_i, dtype=np.int64)
    i_grid = np.arange(NI, dtype=np.int64)[:, None]   # [NI, 1]

    ht = np.stack([h_i, t_i], axis=1).astype(np.int32)    # [B, 2]
    rx = np.asarray(r_i, dtype=np.int32).reshape(B, 1)

    def tileize(a):
        C = a.shape[1]
        return np.ascontiguousarray(
            a.reshape(T, P, C).transpose(1, 0, 2).reshape(P, T * C))

    in_maps = []
    for core in range(N_CORES):
        sl = slice(core * BL, (core + 1) * BL)
        dl = d[sl]                                     # [BL]
        m = ((i_grid >= dl[None, :]) & (i_grid <= dl[None, :] + 4))  # [NI, BL]
        maskT = np.ascontiguousarray(
            m.reshape(IC, P, BL).transpose(1, 0, 2).reshape(P, IC * BL)
        ).astype(ml_dtypes.bfloat16)
        in_maps.append(dict(
            embEp=embEp,
            embRp=embRp,
            csF=csF,
            csT=csT,
            maskT=maskT,
            ht_idx=tileize(ht[sl]),
            r_idx=tileize(rx[sl]),
        ))
    return in_maps


def build_nc():
    nc = bacc.Bacc(
        "TRN2",
        target_bir_lowering=False,
        debug=False,
        enable_asserts=False,
        num_devices=N_CORES,
    )
    ins = dict(
        embEp=nc.dram_tensor("embEp", [N_ENTITY, DD], BF16,
                             kind="ExternalInput").ap(),
        embRp=nc.dram_tensor("embRp", [N_RELATION, DD], BF16,
                             kind="ExternalInput").ap(),
        csF=nc.dram_tensor("csF", [P, IC * DD], BF16,
                           kind="ExternalInput").ap(),
        csT=nc.dram_tensor("csT", [P, JC * NI], BF16,
                           kind="ExternalInput").ap(),
        maskT=nc.dram_tensor("maskT", [P, IC * BL], BF16,
                             kind="ExternalInput").ap(),
        ht_idx=nc.dram_tensor("ht_idx", [P, T * 2], I32,
                              kind="ExternalInput").ap(),
        r_idx=nc.dram_tensor("r_idx", [P, T], I32,
                             kind="ExternalInput").ap(),
    )
    outs = dict(
        out=nc.dram_tensor("out", [P, T], F32, kind="ExternalOutput").ap(),
    )
    with tile.TileContext(nc) as tc:
        _emit(tc, outs, ins)
    nc.compile()
    return nc


_NC_CACHE = {}


def kernel(h_i, t_i, r_i, d_i, emb_E_real, emb_E_img, emb_R_real, emb_R_img,
           time_table, _want_results=False, _trace=False):
    in_maps = _host_prep(h_i, t_i, r_i, d_i, emb_E_real, emb_E_img,
                         emb_R_real, emb_R_img, time_table)
    if "nc" not in _NC_CACHE:
        _NC_CACHE["nc"] = build_nc()
    nc = _NC_CACHE["nc"]
    res = run_bass_kernel_spmd(
        nc, in_maps, core_ids=list(range(N_CORES)), trace=_trace)
    out = np.empty((B,), np.float32)
    for core in range(N_CORES):
        o = res.results[core]["out"]  # [P, T]
        out[core * BL:(core + 1) * BL] = np.asarray(o).T.reshape(BL)
    if _want_results:
        return out, res
    return out


# revision 6
# speedup vs baseline: 2.1754x; 1.1662x over previous
"""ContxE-style temporal KG embedding scoring kernel for Trainium2 (Bass/Tile).

Contract: kernel(**inputs) takes FULL unsharded numpy inputs and returns the
FULL [B] float32 output. Internally shards the batch across 8 NeuronCores
(data-parallel, tables replicated) and runs a Bass/Tile kernel via
run_bass_kernel_spmd.

Math (per batch element b, window W=5, D=512):
  idx[b,w] = d[b]-(4-w), clamped: negatives -> 365
  c/s[b,w,:] = cos/sin(time_table[idx[b,w]])
  h_real = hr*c - hi*s ; h_img = hr*s + hi*c   (same for t)
  4 attention softmaxes over W of <r, rotated>, then weighted sums,
  out = sum|y_r + rr - z_r| + sum|y_i + ri + z_i|

Device-side strategy (per core, B_loc=2048, 4 blocks of 512):
  The two per-element contractions run on the TensorEngine against the
  (small, replicated) extended cos|sin table rather than on DVE:
    phase A:  V[i,b] = <U_ty[b,:], cs_ext[i,:]> for ALL 384 padded table
              rows i as a matmul (stationary = transposed cs table,
              moving = U^T).  The 5 window logits are V[day..day+4, b].
    masked exp:  E = exp(V) * mask  (mask[i,b] = day_b <= i <= day_b+4,
              host-precomputed) gives softmax numerators already in
              [i, b] layout -- no gather/scatter.
    phase B:  CSS[b,:] = E.T @ cs_ext (+ ones column for the softmax
              denominator D), landing back in [b, d] layout; the 1/D
              normalization is folded into the ACT PSUM->SBUF copy as a
              per-partition scale.
  U^T ([d', b] layout) is built from embedding factors transposed via a
  DRAM round-trip with xbar DMA-transpose. Embedding gathers use paired
  [real|img] bf16 rows (one 2KB indirect-DMA row per entity).
"""

import sys

if "/opt/trn_rl_repo" not in sys.path:
    sys.path.insert(0, "/opt/trn_rl_repo")

import numpy as np
import ml_dtypes

import concourse.bass as bass
import concourse.bacc as bacc
import concourse.tile as tile
from concourse import mybir
from concourse.bass_utils import run_bass_kernel_spmd

N_CORES = 8
B = 16384
BL = B // N_CORES          # 2048 per core
P = 128
T = BL // P                # 16 tiles of 128 per core
D = 512
DD = 2 * D                 # 1024 (cos|sin pair width)
W = 5
N_ENTITY = 100000
N_RELATION = 256
PAD_DAY = 365
NI = 384                   # padded extended-table rows (370 used)
IC = NI // P               # 3 i-chunks
JC = DD // P               # 8 d'-chunks
BLK = 512                  # batch block
NBLK = BL // BLK           # 4
SUB = BLK // P             # 4 sub-tiles of 128 per block

F32 = mybir.dt.float32
BF16 = mybir.dt.bfloat16
I32 = mybir.dt.int32

AF = mybir.ActivationFunctionType
OP = mybir.AluOpType


from concourse._compat import with_exitstack


@with_exitstack
def _emit(ctx, tc, outs, ins):
    """Emit the per-core program. outs/ins are dicts of DRAM APs."""
    nc = tc.nc
    embEp = ins["embEp"]      # [N_ENTITY, 1024] bf16  ([real|img] paired rows)
    embRp = ins["embRp"]      # [N_RELATION, 1024] bf16
    csF_d = ins["csF"]        # [128, IC*DD]  bf16  forward ext table, chunked
    csT_d = ins["csT"]        # [128, JC*NI]  bf16  transposed ext table
    mask_d = ins["maskT"]     # [128, IC*BL]  bf16  window mask [i, b]
    ht_idx = ins["ht_idx"]    # [P, T*2] i32  (h, t per tile col)
    r_idx = ins["r_idx"]      # [P, T]   i32
    out = outs["out"]         # [P, T] f32

    singles = ctx.enter_context(tc.tile_pool(name="singles", bufs=1))
    gpool = ctx.enter_context(tc.tile_pool(name="g", bufs=2))
    tpool = ctx.enter_context(tc.tile_pool(name="t", bufs=1))
    upool = ctx.enter_context(tc.tile_pool(name="u", bufs=1))
    epool = ctx.enter_context(tc.tile_pool(name="e", bufs=2))
    apool = ctx.enter_context(tc.tile_pool(name="a", bufs=2))
    wpool = ctx.enter_context(tc.tile_pool(name="w", bufs=1))
    vpsum = ctx.enter_context(tc.tile_pool(name="vps", bufs=1, space="PSUM"))
    cpsum = ctx.enter_context(tc.tile_pool(name="cps", bufs=2, space="PSUM"))
    dpsum = ctx.enter_context(tc.tile_pool(name="dps", bufs=2, space="PSUM"))
    dram = ctx.enter_context(tc.tile_pool(name="dram", bufs=2, space="DRAM"))

    # --- resident tables / indices ---
    csF = singles.tile([P, IC, DD], BF16)    # csF[p,k,:] = cs_pad[k*128+p,:]
    csT = singles.tile([P, JC, NI], BF16)    # csT[p,j,i] = cs_pad[i,j*128+p]
    mask = singles.tile([P, IC, BL], BF16)   # mask[p,k,b]
    sb_ht = singles.tile([P, T * 2], I32)
    sb_r = singles.tile([P, T], I32)
    ones = singles.tile([P, 1], BF16)
    out_acc = singles.tile([P, T], F32)
    nc.sync.dma_start(csF[:], csF_d.rearrange("p (k n) -> p k n", k=IC))
    nc.sync.dma_start(csT[:], csT_d.rearrange("p (j n) -> p j n", j=JC))
    nc.sync.dma_start(mask[:], mask_d.rearrange("p (k n) -> p k n", k=IC))
    nc.sync.dma_start(sb_ht[:], ht_idx[:])
    nc.sync.dma_start(sb_r[:], r_idx[:])
    nc.vector.memset(ones[:], 1.0)

    for blk in range(NBLK):
        # ---- gathers: paired [real|img] rows ----
        hp = gpool.tile([P, SUB, DD], BF16, tag="hp")
        tp = gpool.tile([P, SUB, DD], BF16, tag="tp")
        rp = gpool.tile([P, SUB, DD], BF16, tag="rp")
        for st in range(SUB):
            t_g = blk * SUB + st
            nc.gpsimd.indirect_dma_start(
                out=hp[:, st, :], out_offset=None, in_=embEp[:],
                in_offset=bass.IndirectOffsetOnAxis(
                    ap=sb_ht[:, 2 * t_g: 2 * t_g + 1], axis=0))
            nc.gpsimd.indirect_dma_start(
                out=tp[:, st, :], out_offset=None, in_=embEp[:],
                in_offset=bass.IndirectOffsetOnAxis(
                    ap=sb_ht[:, 2 * t_g + 1: 2 * t_g + 2], axis=0))
            nc.gpsimd.indirect_dma_start(
                out=rp[:, st, :], out_offset=None, in_=embRp[:],
                in_offset=bass.IndirectOffsetOnAxis(
                    ap=sb_r[:, t_g: t_g + 1], axis=0))

        # ---- transpose factors via DRAM round-trip + xbar transpose ----
        hs = dram.tile([BLK, DD], BF16, tag="hs")
        ts_ = dram.tile([BLK, DD], BF16, tag="ts")
        rs = dram.tile([BLK, DD], BF16, tag="rs")
        nc.sync.dma_start(hs.rearrange("(st p) d -> p st d", p=P), hp[:])
        nc.sync.dma_start(ts_.rearrange("(st p) d -> p st d", p=P), tp[:])
        nc.sync.dma_start(rs.rearrange("(st p) d -> p st d", p=P), rp[:])
        hT = tpool.tile([P, JC, BLK], BF16, tag="hT")
        tT = tpool.tile([P, JC, BLK], BF16, tag="tT")
        rT = tpool.tile([P, JC, BLK], BF16, tag="rT")
        nc.sync.dma_start_transpose(hT[:], hs[:])
        nc.sync.dma_start_transpose(tT[:], ts_[:])
        nc.sync.dma_start_transpose(rT[:], rs[:])

        # ---- U^T build: U[p, ty, j, b] = U_ty[d'=j*128+p, b] ----
        # ty0 = [rr*hr | -rr*hi], ty1 = [ri*hi | ri*hr],
        # ty2 = [rr*tr | -rr*ti], ty3 = [ri*ti | ri*tr]
        U = upool.tile([P, 4, JC, BLK], BF16, tag="U")
        nr = wpool.tile([P, JC // 2, BLK], BF16, tag="nr")
        nc.vector.tensor_scalar(out=nr[:], in0=rT[:, 0:4, :], scalar1=-1.0,
                                scalar2=None, op0=OP.mult)
        nc.vector.tensor_tensor(out=U[:, 0, 0:4, :], in0=rT[:, 0:4, :],
                                in1=hT[:, 0:4, :], op=OP.mult)
        nc.vector.tensor_tensor(out=U[:, 0, 4:8, :], in0=nr[:],
                                in1=hT[:, 4:8, :], op=OP.mult)
        nc.vector.tensor_tensor(out=U[:, 1, 0:4, :], in0=rT[:, 4:8, :],
                                in1=hT[:, 4:8, :], op=OP.mult)
        nc.vector.tensor_tensor(out=U[:, 1, 4:8, :], in0=rT[:, 4:8, :],
                                in1=hT[:, 0:4, :], op=OP.mult)
        nc.vector.tensor_tensor(out=U[:, 2, 0:4, :], in0=rT[:, 0:4, :],
                                in1=tT[:, 0:4, :], op=OP.mult)
        nc.vector.tensor_tensor(out=U[:, 2, 4:8, :], in0=nr[:],
                                in1=tT[:, 4:8, :], op=OP.mult)
        nc.vector.tensor_tensor(out=U[:, 3, 0:4, :], in0=rT[:, 4:8, :],
                                in1=tT[:, 4:8, :], op=OP.mult)
        nc.vector.tensor_tensor(out=U[:, 3, 4:8, :], in0=rT[:, 4:8, :],
                                in1=tT[:, 0:4, :], op=OP.mult)

        # ---- phase A: V[i,b] per i-chunk, then E = exp(V)*mask ----
        E = epool.tile([P, 4, IC, BLK], BF16, tag="E")
        for k in range(IC):
            for tp2 in range(2):           # ty pairs share stationary loads
                vts = vpsum.tile([P, 2, BLK], F32, tag="vts")
                for j in range(JC):
                    lhsT = csT[:, j, k * P:(k + 1) * P]
                    for tyh in range(2):
                        ty = tp2 * 2 + tyh
                        nc.tensor.matmul(
                            vts[:, tyh, :], lhsT=lhsT, rhs=U[:, ty, j, :],
                            start=(j == 0), stop=(j == JC - 1))
                for tyh in range(2):
                    ty = tp2 * 2 + tyh
                    nc.scalar.activation(E[:, ty, k, :], vts[:, tyh, :], AF.Exp)
                    nc.vector.tensor_tensor(
                        out=E[:, ty, k, :], in0=E[:, ty, k, :],
                        in1=mask[:, k, blk * BLK:(blk + 1) * BLK], op=OP.mult)

        # ---- phase B + C per 128-row sub-tile ----
        for s in range(SUB):
            bs = slice(s * P, (s + 1) * P)
            dps = dpsum.tile([P, 4], F32, tag="dps")
            A = apool.tile([P, 4, DD], BF16, tag="A")
            rd = wpool.tile([P, 4], F32, tag="rd")
            css = []
            for ty in range(4):
                cps = cpsum.tile([P, DD], F32, tag="cps")
                swap = ty in (1, 3)   # store CSS as [As|Ac] for img types
                for k in range(IC):
                    st_, sp_ = (k == 0), (k == IC - 1)
                    lhsT = E[:, ty, k, bs]
                    lo = csF[:, k, D:DD] if swap else csF[:, k, 0:D]
                    hi = csF[:, k, 0:D] if swap else csF[:, k, D:DD]
                    nc.tensor.matmul(cps[:, 0:D], lhsT=lhsT, rhs=lo,
                                     start=st_, stop=sp_)
                    nc.tensor.matmul(cps[:, D:DD], lhsT=lhsT, rhs=hi,
                                     start=st_, stop=sp_)
                    nc.tensor.matmul(dps[:, ty:ty + 1], lhsT=lhsT,
                                     rhs=ones[:, 0:1], start=st_, stop=sp_)
                css.append(cps)
            nc.vector.reciprocal(rd[:], dps[:])
            for ty in range(4):
                nc.scalar.activation(A[:, ty, :], css[ty][:], AF.Copy,
                                     scale=rd[:, ty:ty + 1])

            # recombine in [b, d] layout
            # G = [hr*A0c | hi*A0s | hr*A1s | hi*A1c], H same with t/A2/A3
            G = wpool.tile([P, 2, DD], BF16, tag="G")
            H = wpool.tile([P, 2, DD], BF16, tag="H")
            nc.vector.tensor_tensor(
                out=G[:], in0=hp[:, s, None, :].broadcast_to([P, 2, DD]),
                in1=A[:, 0:2, :], op=OP.mult)
            nc.vector.tensor_tensor(
                out=H[:], in0=tp[:, s, None, :].broadcast_to([P, 2, DD]),
                in1=A[:, 2:4, :], op=OP.mult)
            yr = wpool.tile([P, D], BF16, tag="yr")
            yi = wpool.tile([P, D], BF16, tag="yi")
            zr = wpool.tile([P, D], BF16, tag="zr")
            zi = wpool.tile([P, D], BF16, tag="zi")
            nc.vector.tensor_tensor(out=yr[:], in0=G[:, 0, 0:D],
                                    in1=G[:, 0, D:DD], op=OP.subtract)
            nc.vector.tensor_tensor(out=yi[:], in0=G[:, 1, 0:D],
                                    in1=G[:, 1, D:DD], op=OP.add)
            nc.vector.tensor_tensor(out=zr[:], in0=H[:, 0, 0:D],
                                    in1=H[:, 0, D:DD], op=OP.subtract)
            nc.vector.tensor_tensor(out=zi[:], in0=H[:, 1, 0:D],
                                    in1=H[:, 1, D:DD], op=OP.add)
            f1 = wpool.tile([P, D], BF16, tag="f1")
            f2 = wpool.tile([P, D], BF16, tag="f2")
            o_r = wpool.tile([P, 1], F32, tag="o_r")
            o_i = wpool.tile([P, 1], F32, tag="o_i")
            dm = wpool.tile([P, D], BF16, tag="dm")
            # f1 = yr - zr + rr ; f2 = yi + zi + ri
            nc.vector.scalar_tensor_tensor(
                out=f1[:], in0=zr[:], scalar=-1.0, in1=yr[:],
                op0=OP.mult, op1=OP.add)
            nc.vector.tensor_tensor(out=f1[:], in0=f1[:], in1=rp[:, s, 0:D],
                                    op=OP.add)
            nc.vector.tensor_tensor(out=f2[:], in0=yi[:], in1=zi[:], op=OP.add)
            nc.vector.tensor_tensor(out=f2[:], in0=f2[:], in1=rp[:, s, D:DD],
                                    op=OP.add)
            nc.scalar.activation(dm[:], f1[:], AF.Abs, accum_out=o_r[:])
            nc.scalar.activation(dm[:], f2[:], AF.Abs, accum_out=o_i[:])
            nc.vector.tensor_tensor(
                out=out_acc[:, blk * SUB + s: blk * SUB + s + 1],
                in0=o_r[:], in1=o_i[:], op=OP.add)

    nc.sync.dma_start(out[:], out_acc[:])


def _host_prep(h_i, t_i, r_i, d_i, emb_E_real, emb_E_img, emb_R_real,
               emb_R_img, time_table):
    """Host-side layout prep (table packing / index manipulation only)."""
    embEp = np.ascontiguousarray(
        np.concatenate([emb_E_real, emb_E_img], axis=1)).astype(
            ml_dtypes.bfloat16)                       # [N_ENTITY, 1024]
    embRp = np.ascontiguousarray(
        np.concatenate([emb_R_real, emb_R_img], axis=1)).astype(
            ml_dtypes.bfloat16)                       # [N_RELATION, 1024]

    tt = np.asarray(time_table, dtype=np.float32)
    cs = np.concatenate([np.cos(tt), np.sin(tt)], axis=1)  # [367, 1024]
    cs_pad = np.zeros((NI, DD), np.float32)
    cs_pad[0:4] = cs[PAD_DAY]          # prefix rows implement neg-idx clamp
    cs_pad[4:370] = cs[0:366]
    # forward table, chunked for SBUF [128, IC, DD]
    csF = np.ascontiguousarray(
        cs_pad.reshape(IC, P, DD).transpose(1, 0, 2).reshape(P, IC * DD)
    ).astype(ml_dtypes.bfloat16)
    # transposed table for SBUF [128, JC, NI]: csT[p, j, i] = cs_pad[i, j*128+p]
    csT = np.ascontiguousarray(
        cs_pad.T.reshape(JC, P, NI).transpose(1, 0, 2).reshape(P, JC * NI)
    ).astype(ml_dtypes.bfloat16)

    d = np.asarray(d_i, dtype=np.int64)
    i_grid = np.arange(NI, dtype=np.int64)[:, None]   # [NI, 1]

    ht = np.stack([h_i, t_i], axis=1).astype(np.int32)    # [B, 2]
    rx = np.asarray(r_i, dtype=np.int32).reshape(B, 1)

    def tileize(a):
        C = a.shape[1]
        return np.ascontiguousarray(
            a.reshape(T, P, C).transpose(1, 0, 2).reshape(P, T * C))

    in_maps = []
    for core in range(N_CORES):
        sl = slice(core * BL, (core + 1) * BL)
        dl = d[sl]                                     # [BL]
        m = ((i_grid >= dl[None, :]) & (i_grid <= dl[None, :] + 4))  # [NI, BL]
        maskT = np.ascontiguousarray(
            m.reshape(IC, P, BL).transpose(1, 0, 2).reshape(P, IC * BL)
        ).astype(ml_dtypes.bfloat16)
        in_maps.append(dict(
            embEp=embEp,
            embRp=embRp,
            csF=csF,
            csT=csT,
            maskT=maskT,
            ht_idx=tileize(ht[sl]),
            r_idx=tileize(rx[sl]),
        ))
    return in_maps


def build_nc():
    nc = bacc.Bacc(
        "TRN2",
        target_bir_lowering=False,
        debug=False,
        enable_asserts=False,
        num_devices=N_CORES,
    )
    ins = dict(
        embEp=nc.dram_tensor("embEp", [N_ENTITY, DD], BF16,
                             kind="ExternalInput").ap(),
        embRp=nc.dram_tensor("embRp", [N_RELATION, DD], BF16,
                             kind="ExternalInput").ap(),
        csF=nc.dram_tensor("csF", [P, IC * DD], BF16,
                           kind="ExternalInput").ap(),
        csT=nc.dram_tensor("csT", [P, JC * NI], BF16,
                           kind="ExternalInput").ap(),
        maskT=nc.dram_tensor("maskT", [P, IC * BL], BF16,
                             kind="ExternalInput").ap(),
        ht_idx=nc.dram_tensor("ht_idx", [P, T * 2], I32,
                              kind="ExternalInput").ap(),
        r_idx=nc.dram_tensor("r_idx", [P, T], I32,
                             kind="ExternalInput").ap(),
    )
    outs = dict(
        out=nc.dram_tensor("out", [P, T], F32, kind="ExternalOutput").ap(),
    )
    with tile.TileContext(nc) as tc:
        _emit(tc, outs, ins)
    nc.compile()
    return nc


_NC_CACHE = {}


def kernel(h_i, t_i, r_i, d_i, emb_E_real, emb_E_img, emb_R_real, emb_R_img,
           time_table, _want_results=False, _trace=False):
    in_maps = _host_prep(h_i, t_i, r_i, d_i, emb_E_real, emb_E_img,
                         emb_R_real, emb_R_img, time_table)
    if "nc" not in _NC_CACHE:
        _NC_CACHE["nc"] = build_nc()
    nc = _NC_CACHE["nc"]
    res = run_bass_kernel_spmd(
        nc, in_maps, core_ids=list(range(N_CORES)), trace=_trace)
    out = np.empty((B,), np.float32)
    for core in range(N_CORES):
        o = res.results[core]["out"]  # [P, T]
        out[core * BL:(core + 1) * BL] = np.asarray(o).T.reshape(BL)
    if _want_results:
        return out, res
    return out


# revision 11
# speedup vs baseline: 2.4327x; 1.1183x over previous
"""ContxE-style temporal KG embedding scoring kernel for Trainium2 (Bass/Tile).

Contract: kernel(**inputs) takes FULL unsharded numpy inputs and returns the
FULL [B] float32 output. Internally shards the batch across 8 NeuronCores
(data-parallel, tables replicated) and runs a Bass/Tile kernel via
run_bass_kernel_spmd.

Math (per batch element b, window W=5, D=512):
  idx[b,w] = d[b]-(4-w), clamped: negatives -> 365
  c/s[b,w,:] = cos/sin(time_table[idx[b,w]])
  h_real = hr*c - hi*s ; h_img = hr*s + hi*c   (same for t)
  4 attention softmaxes over W of <r, rotated>, then weighted sums,
  out = sum|y_r + rr - z_r| + sum|y_i + ri + z_i|

Device-side strategy (per core, B_loc=2048, 4 blocks of 512):
  The two per-element contractions run on the TensorEngine against the
  (small, replicated) extended cos|sin table rather than on DVE:
    phase A:  V[i,b] = <U_ty[b,:], cs_ext[i,:]> for ALL 384 padded table
              rows i as a matmul (stationary = transposed cs table,
              moving = U^T).  The 5 window logits are V[day..day+4, b].
    masked exp:  E = exp(V) * mask  (mask[i,b] = day_b <= i <= day_b+4,
              host-precomputed) gives softmax numerators already in
              [i, b] layout -- no gather/scatter.
    phase B:  CSS[b,:] = E.T @ cs_ext (+ ones column for the softmax
              denominator D), landing back in [b, d] layout; the 1/D
              normalization is folded into the ACT PSUM->SBUF copy as a
              per-partition scale.
  U^T ([d', b] layout) is built from embedding factors transposed via a
  DRAM round-trip with xbar DMA-transpose. Embedding gathers use paired
  [real|img] bf16 rows (one 2KB indirect-DMA row per entity).
"""

import sys

if "/opt/trn_rl_repo" not in sys.path:
    sys.path.insert(0, "/opt/trn_rl_repo")

import numpy as np
import ml_dtypes

import concourse.bass as bass
import concourse.bacc as bacc
import concourse.tile as tile
from concourse import mybir
from concourse.bass_utils import run_bass_kernel_spmd

N_CORES = 8
B = 16384
BL = B // N_CORES          # 2048 per core
P = 128
T = BL // P                # 16 tiles of 128 per core
D = 512
DD = 2 * D                 # 1024 (cos|sin pair width)
W = 5
N_ENTITY = 100000
N_RELATION = 256
PAD_DAY = 365
NI = 384                   # padded extended-table rows (370 used)
IC = NI // P               # 3 i-chunks
JC = DD // P               # 8 d'-chunks
BLK = 512                  # batch block
NBLK = BL // BLK           # 4
SUB = BLK // P             # 4 sub-tiles of 128 per block

F32 = mybir.dt.float32
BF16 = mybir.dt.bfloat16
I32 = mybir.dt.int32

# Static active-chunk schedule for the day-sorted batch.  With d_i ~
# uniform[0,366) and 2048 elements per core sorted by day, sub-tile s
# (128 elements) spans days ~[22.875*s, 22.875*(s+1)] with quantile
# sigma ~4 days; the sets below include >=5-sigma margins.  A host-side
# check patches the (essentially impossible) violating elements.
SB_SETS = [
    (0,), (0,), (0,), (0,),
    (0, 1), (0, 1), (0, 1), (0, 1),
    (1,), (1,), (1, 2), (1, 2),
    (1, 2), (2,), (2,), (2,),
]
KA_SETS = [
    tuple(sorted(set(k for s in range(b * SUB, (b + 1) * SUB)
                 for k in SB_SETS[s]))) for b in range(NBLK)
]

AF = mybir.ActivationFunctionType
OP = mybir.AluOpType


from concourse._compat import with_exitstack


@with_exitstack
def _emit(ctx, tc, outs, ins):
    """Emit the per-core program. outs/ins are dicts of DRAM APs."""
    nc = tc.nc
    embEp = ins["embEp"]      # [N_ENTITY, 1024] bf16  ([real|img] paired rows)
    embRp = ins["embRp"]      # [N_RELATION, 1024] bf16
    csF_d = ins["csF"]        # [128, IC*DD]  bf16  forward ext table, chunked
    csT_d = ins["csT"]        # [128, JC*NI]  bf16  transposed ext table
    mask_d = ins["maskT"]     # [128, IC*BL]  bf16  window mask [i, b]
    ht_idx = ins["ht_idx"]    # [P, T*2] i32  (h, t per tile col)
    r_idx = ins["r_idx"]      # [P, T]   i32
    out = outs["out"]         # [P, T] f32

    singles = ctx.enter_context(tc.tile_pool(name="singles", bufs=1))
    gpool = ctx.enter_context(tc.tile_pool(name="g", bufs=2))
    tpool = ctx.enter_context(tc.tile_pool(name="t", bufs=1))
    upool = ctx.enter_context(tc.tile_pool(name="u", bufs=1))
    epool = ctx.enter_context(tc.tile_pool(name="e", bufs=2))
    apool = ctx.enter_context(tc.tile_pool(name="a", bufs=2))
    wpool = ctx.enter_context(tc.tile_pool(name="w", bufs=1))
    vpsum = ctx.enter_context(tc.tile_pool(name="vps", bufs=1, space="PSUM"))
    cpsum = ctx.enter_context(tc.tile_pool(name="cps", bufs=2, space="PSUM"))
    dpsum = ctx.enter_context(tc.tile_pool(name="dps", bufs=2, space="PSUM"))
    dram = ctx.enter_context(tc.tile_pool(name="dram", bufs=2, space="DRAM"))

    # --- resident tables / indices ---
    csF = singles.tile([P, IC, DD], BF16)    # csF[p,k,:] = cs_pad[k*128+p,:]
    csT = singles.tile([P, JC, NI], BF16)    # csT[p,j,i] = cs_pad[i,j*128+p]
    mask = singles.tile([P, IC, BL], BF16)   # mask[p,k,b]
    sb_ht = singles.tile([P, T * 2], I32)
    sb_r = singles.tile([P, T], I32)
    ones = singles.tile([P, 1], BF16)
    out_acc = singles.tile([P, T], F32)
    nc.sync.dma_start(csF[:], csF_d.rearrange("p (k n) -> p k n", k=IC))
    nc.sync.dma_start(csT[:], csT_d.rearrange("p (j n) -> p j n", j=JC))
    nc.sync.dma_start(mask[:], mask_d.rearrange("p (k n) -> p k n", k=IC))
    nc.sync.dma_start(sb_ht[:], ht_idx[:])
    nc.sync.dma_start(sb_r[:], r_idx[:])
    nc.vector.memset(ones[:], 1.0)

    for blk in range(NBLK):
        # ---- gathers: paired [real|img] rows ----
        hp = gpool.tile([P, SUB, DD], BF16, tag="hp")
        tp = gpool.tile([P, SUB, DD], BF16, tag="tp")
        rp = gpool.tile([P, SUB, DD], BF16, tag="rp")
        for st in range(SUB):
            t_g = blk * SUB + st
            nc.gpsimd.indirect_dma_start(
                out=hp[:, st, :], out_offset=None, in_=embEp[:],
                in_offset=bass.IndirectOffsetOnAxis(
                    ap=sb_ht[:, 2 * t_g: 2 * t_g + 1], axis=0))
            nc.gpsimd.indirect_dma_start(
                out=tp[:, st, :], out_offset=None, in_=embEp[:],
                in_offset=bass.IndirectOffsetOnAxis(
                    ap=sb_ht[:, 2 * t_g + 1: 2 * t_g + 2], axis=0))
            nc.gpsimd.indirect_dma_start(
                out=rp[:, st, :], out_offset=None, in_=embRp[:],
                in_offset=bass.IndirectOffsetOnAxis(
                    ap=sb_r[:, t_g: t_g + 1], axis=0))

        # ---- transpose factors via DRAM round-trip + xbar transpose ----
        hs = dram.tile([BLK, DD], BF16, tag="hs")
        ts_ = dram.tile([BLK, DD], BF16, tag="ts")
        rs = dram.tile([BLK, DD], BF16, tag="rs")
        nc.sync.dma_start(hs.rearrange("(st p) d -> p st d", p=P), hp[:])
        nc.sync.dma_start(ts_.rearrange("(st p) d -> p st d", p=P), tp[:])
        nc.sync.dma_start(rs.rearrange("(st p) d -> p st d", p=P), rp[:])
        hT = tpool.tile([P, JC, BLK], BF16, tag="hT")
        tT = tpool.tile([P, JC, BLK], BF16, tag="tT")
        rT = tpool.tile([P, JC, BLK], BF16, tag="rT")
        nc.sync.dma_start_transpose(hT[:], hs[:])
        nc.sync.dma_start_transpose(tT[:], ts_[:])
        nc.sync.dma_start_transpose(rT[:], rs[:])

        # ---- U^T build: U[p, ty, j, b] = U_ty[d'=j*128+p, b] ----
        # ty0 = [rr*hr | -rr*hi], ty1 = [ri*hi | ri*hr],
        # ty2 = [rr*tr | -rr*ti], ty3 = [ri*ti | ri*tr]
        U = upool.tile([P, 4, JC, BLK], BF16, tag="U")
        nr = wpool.tile([P, JC // 2, BLK], BF16, tag="nr")
        nc.vector.tensor_scalar(out=nr[:], in0=rT[:, 0:4, :], scalar1=-1.0,
                                scalar2=None, op0=OP.mult)
        nc.vector.tensor_tensor(out=U[:, 0, 0:4, :], in0=rT[:, 0:4, :],
                                in1=hT[:, 0:4, :], op=OP.mult)
        nc.vector.tensor_tensor(out=U[:, 0, 4:8, :], in0=nr[:],
                                in1=hT[:, 4:8, :], op=OP.mult)
        nc.vector.tensor_tensor(out=U[:, 1, 0:4, :], in0=rT[:, 4:8, :],
                                in1=hT[:, 4:8, :], op=OP.mult)
        nc.vector.tensor_tensor(out=U[:, 1, 4:8, :], in0=rT[:, 4:8, :],
                                in1=hT[:, 0:4, :], op=OP.mult)
        nc.vector.tensor_tensor(out=U[:, 2, 0:4, :], in0=rT[:, 0:4, :],
                                in1=tT[:, 0:4, :], op=OP.mult)
        nc.vector.tensor_tensor(out=U[:, 2, 4:8, :], in0=nr[:],
                                in1=tT[:, 4:8, :], op=OP.mult)
        nc.vector.tensor_tensor(out=U[:, 3, 0:4, :], in0=rT[:, 4:8, :],
                                in1=tT[:, 4:8, :], op=OP.mult)
        nc.vector.tensor_tensor(out=U[:, 3, 4:8, :], in0=rT[:, 4:8, :],
                                in1=tT[:, 0:4, :], op=OP.mult)

        # ---- phase A: V[i,b] per active i-chunk, then E = exp(V)*mask ----
        E = epool.tile([P, 4, IC, BLK], BF16, tag="E")
        for k in KA_SETS[blk]:
            for tp2 in range(2):           # ty pairs share stationary loads
                vts = vpsum.tile([P, 2, BLK], F32, tag="vts")
                for j in range(JC):
                    lhsT = csT[:, j, k * P:(k + 1) * P]
                    for tyh in range(2):
                        ty = tp2 * 2 + tyh
                        nc.tensor.matmul(
                            vts[:, tyh, :], lhsT=lhsT, rhs=U[:, ty, j, :],
                            start=(j == 0), stop=(j == JC - 1))
                for tyh in range(2):
                    ty = tp2 * 2 + tyh
                    nc.scalar.activation(E[:, ty, k, :], vts[:, tyh, :], AF.Exp)
                    nc.vector.tensor_tensor(
                        out=E[:, ty, k, :], in0=E[:, ty, k, :],
                        in1=mask[:, k, blk * BLK:(blk + 1) * BLK], op=OP.mult)

        # ---- phase B + C per 128-row sub-tile ----
        for s in range(SUB):
            bs = slice(s * P, (s + 1) * P)
            dps = dpsum.tile([P, 4], F32, tag="dps")
            A = apool.tile([P, 4, DD], BF16, tag="A")
            rd = wpool.tile([P, 4], F32, tag="rd")
            ks = SB_SETS[blk * SUB + s]
            css = []
            for ty in range(4):
                cps = cpsum.tile([P, DD], F32, tag="cps")
                swap = ty in (1, 3)   # store CSS as [As|Ac] for img types
                for k in ks:
                    st_, sp_ = (k == ks[0]), (k == ks[-1])
                    lhsT = E[:, ty, k, bs]
                    lo = csF[:, k, D:DD] if swap else csF[:, k, 0:D]
                    hi = csF[:, k, 0:D] if swap else csF[:, k, D:DD]
                    nc.tensor.matmul(cps[:, 0:D], lhsT=lhsT, rhs=lo,
                                     start=st_, stop=sp_)
                    nc.tensor.matmul(cps[:, D:DD], lhsT=lhsT, rhs=hi,
                                     start=st_, stop=sp_)
                    nc.tensor.matmul(dps[:, ty:ty + 1], lhsT=lhsT,
                                     rhs=ones[:, 0:1], start=st_, stop=sp_)
                css.append(cps)
            nc.vector.reciprocal(rd[:], dps[:])
            for ty in range(4):
                nc.scalar.activation(A[:, ty, :], css[ty][:], AF.Copy,
                                     scale=rd[:, ty:ty + 1])

            # recombine in [b, d] layout
            # G = [hr*A0c | hi*A0s | hr*A1s | hi*A1c], H same with t/A2/A3
            G = wpool.tile([P, 2, DD], BF16, tag="G")
            H = wpool.tile([P, 2, DD], BF16, tag="H")
            nc.vector.tensor_tensor(
                out=G[:], in0=hp[:, s, None, :].broadcast_to([P, 2, DD]),
                in1=A[:, 0:2, :], op=OP.mult)
            nc.vector.tensor_tensor(
                out=H[:], in0=tp[:, s, None, :].broadcast_to([P, 2, DD]),
                in1=A[:, 2:4, :], op=OP.mult)
            yr = wpool.tile([P, D], BF16, tag="yr")
            yi = wpool.tile([P, D], BF16, tag="yi")
            zr = wpool.tile([P, D], BF16, tag="zr")
            zi = wpool.tile([P, D], BF16, tag="zi")
            nc.vector.tensor_tensor(out=yr[:], in0=G[:, 0, 0:D],
                                    in1=G[:, 0, D:DD], op=OP.subtract)
            nc.vector.tensor_tensor(out=yi[:], in0=G[:, 1, 0:D],
                                    in1=G[:, 1, D:DD], op=OP.add)
            nc.vector.tensor_tensor(out=zr[:], in0=H[:, 0, 0:D],
                                    in1=H[:, 0, D:DD], op=OP.subtract)
            nc.vector.tensor_tensor(out=zi[:], in0=H[:, 1, 0:D],
                                    in1=H[:, 1, D:DD], op=OP.add)
            f1 = wpool.tile([P, D], BF16, tag="f1")
            f2 = wpool.tile([P, D], BF16, tag="f2")
            o_r = wpool.tile([P, 1], F32, tag="o_r")
            o_i = wpool.tile([P, 1], F32, tag="o_i")
            dm = wpool.tile([P, D], BF16, tag="dm")
            # f1 = yr - zr + rr ; f2 = yi + zi + ri
            nc.vector.scalar_tensor_tensor(
                out=f1[:], in0=zr[:], scalar=-1.0, in1=yr[:],
                op0=OP.mult, op1=OP.add)
            nc.vector.tensor_tensor(out=f1[:], in0=f1[:], in1=rp[:, s, 0:D],
                                    op=OP.add)
            nc.vector.tensor_tensor(out=f2[:], in0=yi[:], in1=zi[:], op=OP.add)
            nc.vector.tensor_tensor(out=f2[:], in0=f2[:], in1=rp[:, s, D:DD],
                                    op=OP.add)
            nc.scalar.activation(dm[:], f1[:], AF.Abs, accum_out=o_r[:])
            nc.scalar.activation(dm[:], f2[:], AF.Abs, accum_out=o_i[:])
            nc.vector.tensor_tensor(
                out=out_acc[:, blk * SUB + s: blk * SUB + s + 1],
                in0=o_r[:], in1=o_i[:], op=OP.add)

    nc.sync.dma_start(out[:], out_acc[:])


def _host_prep(h_i, t_i, r_i, d_i, emb_E_real, emb_E_img, emb_R_real,
               emb_R_img, time_table):
    """Host-side layout prep (table packing / index manipulation only)."""
    embEp = np.ascontiguousarray(
        np.concatenate([emb_E_real, emb_E_img], axis=1)).astype(
            ml_dtypes.bfloat16)                       # [N_ENTITY, 1024]
    embRp = np.ascontiguousarray(
        np.concatenate([emb_R_real, emb_R_img], axis=1)).astype(
            ml_dtypes.bfloat16)                       # [N_RELATION, 1024]

    tt = np.asarray(time_table, dtype=np.float32)
    cs = np.concatenate([np.cos(tt), np.sin(tt)], axis=1)  # [367, 1024]
    cs_pad = np.zeros((NI, DD), np.float32)
    cs_pad[0:4] = cs[PAD_DAY]          # prefix rows implement neg-idx clamp
    cs_pad[4:370] = cs[0:366]
    # forward table, chunked for SBUF [128, IC, DD]
    csF = np.ascontiguousarray(
        cs_pad.reshape(IC, P, DD).transpose(1, 0, 2).reshape(P, IC * DD)
    ).astype(ml_dtypes.bfloat16)
    # transposed table for SBUF [128, JC, NI]: csT[p, j, i] = cs_pad[i, j*128+p]
    csT = np.ascontiguousarray(
        cs_pad.T.reshape(JC, P, NI).transpose(1, 0, 2).reshape(P, JC * NI)
    ).astype(ml_dtypes.bfloat16)

    d = np.asarray(d_i, dtype=np.int64)
    i_grid = np.arange(NI, dtype=np.int64)[:, None]   # [NI, 1]

    ht = np.stack([h_i, t_i], axis=1).astype(np.int32)    # [B, 2]
    rx = np.asarray(r_i, dtype=np.int32).reshape(B, 1)

    def tileize(a):
        C = a.shape[1]
        return np.ascontiguousarray(
            a.reshape(T, P, C).transpose(1, 0, 2).reshape(P, T * C))

    in_maps = []
    perms = []
    fallback = []          # original global indices needing host fixup
    for core in range(N_CORES):
        sl = slice(core * BL, (core + 1) * BL)
        perm = np.argsort(d[sl], kind="stable")        # sorted-by-day order
        perms.append(perm)
        dl = d[sl][perm]                               # [BL] sorted
        # safety check: each sub-tile's window rows must fit its static
        # chunk set; collect violating elements for exact host fixup
        ds = dl.reshape(T, P)
        for s in range(T):
            lo, hi = int(ds[s].min()), int(ds[s].max()) + 4
            ok = np.zeros(NI, bool)
            for k in SB_SETS[s]:
                ok[k * P:(k + 1) * P] = True
            if not ok[lo:hi + 1].all():
                bad = np.arange(s * P, (s + 1) * P)
                fallback.extend(core * BL + perm[bad])
        m = ((i_grid >= dl[None, :]) & (i_grid <= dl[None, :] + 4))  # [NI, BL]
        maskT = np.ascontiguousarray(
            m.reshape(IC, P, BL).transpose(1, 0, 2).reshape(P, IC * BL)
        ).astype(ml_dtypes.bfloat16)
        in_maps.append(dict(
            embEp=embEp,
            embRp=embRp,
            csF=csF,
            csT=csT,
            maskT=maskT,
            ht_idx=tileize(ht[sl][perm]),
            r_idx=tileize(rx[sl][perm]),
        ))
    return in_maps, perms, np.asarray(fallback, dtype=np.int64)


def _reference_np(h_i, t_i, r_i, d_i, eR, eI, rR, rI, tt):
    """Exact numpy replica of the reference for rare host fixups."""
    n_day = tt.shape[0] - 2
    idx = d_i[:, None] - np.arange(W - 1, -1, -1)[None, :]
    idx = np.where(idx >= 0, idx, n_day)
    dl = tt[idx]
    s_, c_ = np.sin(dl), np.cos(dl)
    hr, hi = eR[h_i][:, None, :], eI[h_i][:, None, :]
    tr, ti = eR[t_i][:, None, :], eI[t_i][:, None, :]
    h_re, h_im = hr * c_ - hi * s_, hr * s_ + hi * c_
    t_re, t_im = tr * c_ - ti * s_, tr * s_ + ti * c_
    rr, ri = rR[r_i], rI[r_i]

    def soft(lg):
        e = np.exp(lg - lg.max(axis=1, keepdims=True))
        return (e / e.sum(axis=1, keepdims=True))[..., None]

    a_r = soft(np.einsum("bd,bwd->bw", rr, h_re))
    a_i = soft(np.einsum("bd,bwd->bw", ri, h_im))
    b_r = soft(np.einsum("bd,bwd->bw", rr, t_re))
    b_i = soft(np.einsum("bd,bwd->bw", ri, t_im))
    y_r = (a_r * h_re).sum(1)
    y_i = (a_i * h_im).sum(1)
    z_r = (b_r * t_re).sum(1)
    z_i = (b_i * t_im).sum(1)
    return (np.abs(y_r + rr - z_r).sum(1)
            + np.abs(y_i + ri + z_i).sum(1)).astype(np.float32)


def build_nc():
    nc = bacc.Bacc(
        "TRN2",
        target_bir_lowering=False,
        debug=False,
        enable_asserts=False,
        num_devices=N_CORES,
    )
    ins = dict(
        embEp=nc.dram_tensor("embEp", [N_ENTITY, DD], BF16,
                             kind="ExternalInput").ap(),
        embRp=nc.dram_tensor("embRp", [N_RELATION, DD], BF16,
                             kind="ExternalInput").ap(),
        csF=nc.dram_tensor("csF", [P, IC * DD], BF16,
                           kind="ExternalInput").ap(),
        csT=nc.dram_tensor("csT", [P, JC * NI], BF16,
                           kind="ExternalInput").ap(),
        maskT=nc.dram_tensor("maskT", [P, IC * BL], BF16,
                             kind="ExternalInput").ap(),
        ht_idx=nc.dram_tensor("ht_idx", [P, T * 2], I32,
                              kind="ExternalInput").ap(),
        r_idx=nc.dram_tensor("r_idx", [P, T], I32,
                             kind="ExternalInput").ap(),
    )
    outs = dict(
        out=nc.dram_tensor("out", [P, T], F32, kind="ExternalOutput").ap(),
    )
    with tile.TileContext(nc) as tc:
        _emit(tc, outs, ins)
    nc.compile()
    return nc


_NC_CACHE = {}


def kernel(h_i, t_i, r_i, d_i, emb_E_real, emb_E_img, emb_R_real, emb_R_img,
           time_table, _want_results=False, _trace=False):
    in_maps, perms, fallback = _host_prep(
        h_i, t_i, r_i, d_i, emb_E_real, emb_E_img, emb_R_real, emb_R_img,
        time_table)
    if "nc" not in _NC_CACHE:
        _NC_CACHE["nc"] = build_nc()
    nc = _NC_CACHE["nc"]
    res = run_bass_kernel_spmd(
        nc, in_maps, core_ids=list(range(N_CORES)), trace=_trace)
    out = np.empty((B,), np.float32)
    for core in range(N_CORES):
        o = np.asarray(res.results[core]["out"]).T.reshape(BL)  # sorted order
        out[core * BL + perms[core]] = o
    if len(fallback):
        f = np.asarray(fallback)
        out[f] = _reference_np(
            np.asarray(h_i)[f], np.asarray(t_i)[f], np.asarray(r_i)[f],
            np.asarray(d_i)[f], np.asarray(emb_E_real),
            np.asarray(emb_E_img), np.asarray(emb_R_real),
            np.asarray(emb_R_img), np.asarray(time_table, dtype=np.float32))
    if _want_results:
        return out, res
    return out


# revision 15
# speedup vs baseline: 2.5570x; 1.0511x over previous
"""ContxE-style temporal KG embedding scoring kernel for Trainium2 (Bass/Tile).

Contract: kernel(**inputs) takes FULL unsharded numpy inputs and returns the
FULL [B] float32 output. Internally shards the batch across 8 NeuronCores
(data-parallel, tables replicated) and runs a Bass/Tile kernel via
run_bass_kernel_spmd.

Math (per batch element b, window W=5, D=512):
  idx[b,w] = d[b]-(4-w), clamped: negatives -> 365
  c/s[b,w,:] = cos/sin(time_table[idx[b,w]])
  h_real = hr*c - hi*s ; h_img = hr*s + hi*c   (same for t)
  4 attention softmaxes over W of <r, rotated>, then weighted sums,
  out = sum|y_r + rr - z_r| + sum|y_i + ri + z_i|

Device-side strategy (per core, B_loc=2048, 4 blocks of 512):
  The two per-element contractions run on the TensorEngine against the
  (small, replicated) extended cos|sin table rather than on DVE:
    phase A:  V[i,b] = <U_ty[b,:], cs_ext[i,:]> for ALL 384 padded table
              rows i as a matmul (stationary = transposed cs table,
              moving = U^T).  The 5 window logits are V[day..day+4, b].
    masked exp:  E = exp(V) * mask  (mask[i,b] = day_b <= i <= day_b+4,
              host-precomputed) gives softmax numerators already in
              [i, b] layout -- no gather/scatter.
    phase B:  CSS[b,:] = E.T @ cs_ext (+ ones column for the softmax
              denominator D), landing back in [b, d] layout; the 1/D
              normalization is folded into the ACT PSUM->SBUF copy as a
              per-partition scale.
  U^T ([d', b] layout) is built from embedding factors transposed via a
  DRAM round-trip with xbar DMA-transpose. Embedding gathers use paired
  [real|img] bf16 rows (one 2KB indirect-DMA row per entity).
"""

import sys

if "/opt/trn_rl_repo" not in sys.path:
    sys.path.insert(0, "/opt/trn_rl_repo")

import numpy as np
import ml_dtypes

import concourse.bass as bass
import concourse.bacc as bacc
import concourse.tile as tile
from concourse import mybir
from concourse.bass_utils import run_bass_kernel_spmd

N_CORES = 8
B = 16384
BL = B // N_CORES          # 2048 per core
P = 128
T = BL // P                # 16 tiles of 128 per core
D = 512
DD = 2 * D                 # 1024 (cos|sin pair width)
W = 5
N_ENTITY = 100000
N_RELATION = 256
PAD_DAY = 365
NI = 384                   # padded extended-table rows (370 used)
IC = NI // P               # 3 i-chunks
JC = DD // P               # 8 d'-chunks
BLK = 512                  # batch block
NBLK = BL // BLK           # 4
SUB = BLK // P             # 4 sub-tiles of 128 per block

F32 = mybir.dt.float32
BF16 = mybir.dt.bfloat16
I32 = mybir.dt.int32

# Static active-chunk schedule for the day-sorted batch.  With d_i ~
# uniform[0,366) and 2048 elements per core sorted by day, sub-tile s
# (128 elements) spans days ~[22.875*s, 22.875*(s+1)] with quantile
# sigma ~4 days; the sets below include >=5-sigma margins.  A host-side
# check patches the (essentially impossible) violating elements.
SB_SETS = [
    (0,), (0,), (0,), (0,),
    (0, 1), (0, 1), (0, 1), (0, 1),
    (1,), (1,), (1, 2), (1, 2),
    (1, 2), (2,), (2,), (2,),
]
KA_SETS = [
    tuple(sorted(set(k for s in range(b * SUB, (b + 1) * SUB)
                 for k in SB_SETS[s]))) for b in range(NBLK)
]

AF = mybir.ActivationFunctionType
OP = mybir.AluOpType


from concourse._compat import with_exitstack


@with_exitstack
def _emit(ctx, tc, outs, ins):
    """Emit the per-core program. outs/ins are dicts of DRAM APs."""
    nc = tc.nc
    embEp = ins["embEp"]      # [N_ENTITY, 1024] bf16  ([real|img] paired rows)
    embRp = ins["embRp"]      # [N_RELATION, 1024] bf16
    csF_d = ins["csF"]        # [128, IC*DD]  bf16  forward ext table, chunked
    csT_d = ins["csT"]        # [128, JC*NI]  bf16  transposed ext table
    mask_d = ins["maskT"]     # [128, IC*BL]  bf16  window mask [i, b]
    ht_idx = ins["ht_idx"]    # [P, T*2] i32  (h, t per tile col)
    r_idx = ins["r_idx"]      # [P, T]   i32
    out = outs["out"]         # [P, T] f32

    singles = ctx.enter_context(tc.tile_pool(name="singles", bufs=1))
    gpool = ctx.enter_context(tc.tile_pool(name="g", bufs=2))
    tpool = ctx.enter_context(tc.tile_pool(name="t", bufs=2))
    upool = ctx.enter_context(tc.tile_pool(name="u", bufs=2))
    epool = ctx.enter_context(tc.tile_pool(name="e", bufs=2))
    apool = ctx.enter_context(tc.tile_pool(name="a", bufs=1))
    wpool = ctx.enter_context(tc.tile_pool(name="w", bufs=1))
    vpsum = ctx.enter_context(tc.tile_pool(name="vps", bufs=1, space="PSUM"))
    cpsum = ctx.enter_context(tc.tile_pool(name="cps", bufs=2, space="PSUM"))
    dpsum = ctx.enter_context(tc.tile_pool(name="dps", bufs=2, space="PSUM"))
    dram = ctx.enter_context(tc.tile_pool(name="dram", bufs=2, space="DRAM"))

    # --- resident tables / indices ---
    csF = singles.tile([P, IC, DD], BF16)    # csF[p,k,:] = cs_pad[k*128+p,:]
    csT = singles.tile([P, JC, NI], BF16)    # csT[p,j,i] = cs_pad[i,j*128+p]
    mask = singles.tile([P, IC, BL], BF16)   # mask[p,k,b]
    sb_ht = singles.tile([P, T * 2], I32)
    sb_r = singles.tile([P, T], I32)
    ones = singles.tile([P, 1], BF16)
    out_acc = singles.tile([P, T], F32)
    nc.sync.dma_start(csF[:], csF_d.rearrange("p (k n) -> p k n", k=IC))
    nc.sync.dma_start(csT[:], csT_d.rearrange("p (j n) -> p j n", j=JC))
    nc.sync.dma_start(mask[:], mask_d.rearrange("p (k n) -> p k n", k=IC))
    nc.sync.dma_start(sb_ht[:], ht_idx[:])
    nc.sync.dma_start(sb_r[:], r_idx[:])
    nc.vector.memset(ones[:], 1.0)

    for blk in range(NBLK):
        # ---- gathers: paired [real|img] rows ----
        hp = gpool.tile([P, SUB, DD], BF16, tag="hp")
        tp = gpool.tile([P, SUB, DD], BF16, tag="tp")
        rp = gpool.tile([P, SUB, DD], BF16, tag="rp")
        for st in range(SUB):
            t_g = blk * SUB + st
            nc.gpsimd.indirect_dma_start(
                out=hp[:, st, :], out_offset=None, in_=embEp[:],
                in_offset=bass.IndirectOffsetOnAxis(
                    ap=sb_ht[:, 2 * t_g: 2 * t_g + 1], axis=0))
            nc.gpsimd.indirect_dma_start(
                out=tp[:, st, :], out_offset=None, in_=embEp[:],
                in_offset=bass.IndirectOffsetOnAxis(
                    ap=sb_ht[:, 2 * t_g + 1: 2 * t_g + 2], axis=0))
            nc.gpsimd.indirect_dma_start(
                out=rp[:, st, :], out_offset=None, in_=embRp[:],
                in_offset=bass.IndirectOffsetOnAxis(
                    ap=sb_r[:, t_g: t_g + 1], axis=0))

        # ---- transpose factors via DRAM round-trip + ONE xbar transpose ----
        scr = dram.tile([BLK, 3 * DD], BF16, tag="scr")
        nc.sync.dma_start(
            scr[:, 0:DD].rearrange("(st p) d -> p st d", p=P), hp[:])
        nc.sync.dma_start(
            scr[:, DD:2 * DD].rearrange("(st p) d -> p st d", p=P), tp[:])
        nc.sync.dma_start(
            scr[:, 2 * DD:3 * DD].rearrange("(st p) d -> p st d", p=P), rp[:])
        xT = tpool.tile([P, 3 * JC, BLK], BF16, tag="xT")
        nc.sync.dma_start_transpose(xT[:], scr[:])
        hT = xT[:, 0:JC, :]
        tT = xT[:, JC:2 * JC, :]
        rT = xT[:, 2 * JC:3 * JC, :]

        # ---- U^T build in cos/sin j-halves (double-buffered) ----
        # ty0 = [rr*hr | -rr*hi], ty1 = [ri*hi | ri*hr],
        # ty2 = [rr*tr | -rr*ti], ty3 = [ri*ti | ri*tr]
        uh0 = upool.tile([P, 4, JC // 2, BLK], BF16, tag="U")   # j 0..3 (cos)
        uh1 = upool.tile([P, 4, JC // 2, BLK], BF16, tag="U")   # j 4..7 (sin)
        nr = wpool.tile([P, JC // 2, BLK], BF16, tag="nr")
        nc.vector.tensor_scalar(out=nr[:], in0=rT[:, 0:4, :], scalar1=-1.0,
                                scalar2=None, op0=OP.mult)
        nc.vector.tensor_tensor(out=uh0[:, 0], in0=rT[:, 0:4, :],
                                in1=hT[:, 0:4, :], op=OP.mult)
        nc.vector.tensor_tensor(out=uh0[:, 1], in0=rT[:, 4:8, :],
                                in1=hT[:, 4:8, :], op=OP.mult)
        nc.vector.tensor_tensor(out=uh0[:, 2], in0=rT[:, 0:4, :],
                                in1=tT[:, 0:4, :], op=OP.mult)
        nc.vector.tensor_tensor(out=uh0[:, 3], in0=rT[:, 4:8, :],
                                in1=tT[:, 4:8, :], op=OP.mult)
        nc.vector.tensor_tensor(out=uh1[:, 0], in0=nr[:],
                                in1=hT[:, 4:8, :], op=OP.mult)
        nc.vector.tensor_tensor(out=uh1[:, 1], in0=rT[:, 4:8, :],
                                in1=hT[:, 0:4, :], op=OP.mult)
        nc.vector.tensor_tensor(out=uh1[:, 2], in0=nr[:],
                                in1=tT[:, 4:8, :], op=OP.mult)
        nc.vector.tensor_tensor(out=uh1[:, 3], in0=rT[:, 4:8, :],
                                in1=tT[:, 0:4, :], op=OP.mult)

        # ---- phase A: V[i,b] per active i-chunk, then E = exp(V)*mask ----
        # E slot q holds chunk KA_SETS[blk][q]
        E = epool.tile([P, 4, 2, BLK], BF16, tag="E")
        for q, k in enumerate(KA_SETS[blk]):
            for tp2 in range(2):           # ty pairs share stationary loads
                vts = vpsum.tile([P, 2, BLK], F32, tag="vts")
                for j in range(JC):
                    lhsT = csT[:, j, k * P:(k + 1) * P]
                    uh = uh0 if j < JC // 2 else uh1
                    for tyh in range(2):
                        ty = tp2 * 2 + tyh
                        nc.tensor.matmul(
                            vts[:, tyh, :], lhsT=lhsT,
                            rhs=uh[:, ty, j % (JC // 2), :],
                            start=(j == 0), stop=(j == JC - 1))
                for tyh in range(2):
                    ty = tp2 * 2 + tyh
                    nc.scalar.activation(E[:, ty, q, :], vts[:, tyh, :], AF.Exp)
                    nc.vector.tensor_tensor(
                        out=E[:, ty, q, :], in0=E[:, ty, q, :],
                        in1=mask[:, k, blk * BLK:(blk + 1) * BLK], op=OP.mult)

        # ---- phase B + C per 128-row sub-tile ----
        for s in range(SUB):
            bs = slice(s * P, (s + 1) * P)
            dps = dpsum.tile([P, 4], F32, tag="dps")
            A = apool.tile([P, 4, DD], BF16, tag="A")
            rd = wpool.tile([P, 4], F32, tag="rd")
            ks = SB_SETS[blk * SUB + s]
            css = []
            for ty in range(4):
                cps = cpsum.tile([P, DD], F32, tag="cps")
                swap = ty in (1, 3)   # store CSS as [As|Ac] for img types
                for k in ks:
                    st_, sp_ = (k == ks[0]), (k == ks[-1])
                    lhsT = E[:, ty, KA_SETS[blk].index(k), bs]
                    lo = csF[:, k, D:DD] if swap else csF[:, k, 0:D]
                    hi = csF[:, k, 0:D] if swap else csF[:, k, D:DD]
                    nc.tensor.matmul(cps[:, 0:D], lhsT=lhsT, rhs=lo,
                                     start=st_, stop=sp_)
                    nc.tensor.matmul(cps[:, D:DD], lhsT=lhsT, rhs=hi,
                                     start=st_, stop=sp_)
                    nc.tensor.matmul(dps[:, ty:ty + 1], lhsT=lhsT,
                                     rhs=ones[:, 0:1], start=st_, stop=sp_)
                css.append(cps)
            nc.vector.reciprocal(rd[:], dps[:])
            for ty in range(4):
                nc.scalar.activation(A[:, ty, :], css[ty][:], AF.Copy,
                                     scale=rd[:, ty:ty + 1])

            # recombine in [b, d] layout
            # G = [hr*A0c | hi*A0s | hr*A1s | hi*A1c], H same with t/A2/A3
            G = wpool.tile([P, 2, DD], BF16, tag="G")
            H = wpool.tile([P, 2, DD], BF16, tag="H")
            nc.vector.tensor_tensor(
                out=G[:], in0=hp[:, s, None, :].broadcast_to([P, 2, DD]),
                in1=A[:, 0:2, :], op=OP.mult)
            nc.vector.tensor_tensor(
                out=H[:], in0=tp[:, s, None, :].broadcast_to([P, 2, DD]),
                in1=A[:, 2:4, :], op=OP.mult)
            yr = wpool.tile([P, D], BF16, tag="yr")
            yi = wpool.tile([P, D], BF16, tag="yi")
            zr = wpool.tile([P, D], BF16, tag="zr")
            zi = wpool.tile([P, D], BF16, tag="zi")
            nc.vector.tensor_tensor(out=yr[:], in0=G[:, 0, 0:D],
                                    in1=G[:, 0, D:DD], op=OP.subtract)
            nc.vector.tensor_tensor(out=yi[:], in0=G[:, 1, 0:D],
                                    in1=G[:, 1, D:DD], op=OP.add)
            nc.vector.tensor_tensor(out=zr[:], in0=H[:, 0, 0:D],
                                    in1=H[:, 0, D:DD], op=OP.subtract)
            nc.vector.tensor_tensor(out=zi[:], in0=H[:, 1, 0:D],
                                    in1=H[:, 1, D:DD], op=OP.add)
            f1 = wpool.tile([P, D], BF16, tag="f1")
            f2 = wpool.tile([P, D], BF16, tag="f2")
            o_r = wpool.tile([P, 1], F32, tag="o_r")
            o_i = wpool.tile([P, 1], F32, tag="o_i")
            dm = wpool.tile([P, D], BF16, tag="dm")
            # f1 = yr - zr + rr ; f2 = yi + zi + ri
            nc.vector.scalar_tensor_tensor(
                out=f1[:], in0=zr[:], scalar=-1.0, in1=yr[:],
                op0=OP.mult, op1=OP.add)
            nc.vector.tensor_tensor(out=f1[:], in0=f1[:], in1=rp[:, s, 0:D],
                                    op=OP.add)
            nc.vector.tensor_tensor(out=f2[:], in0=yi[:], in1=zi[:], op=OP.add)
            nc.vector.tensor_tensor(out=f2[:], in0=f2[:], in1=rp[:, s, D:DD],
                                    op=OP.add)
            nc.scalar.activation(dm[:], f1[:], AF.Abs, accum_out=o_r[:])
            nc.scalar.activation(dm[:], f2[:], AF.Abs, accum_out=o_i[:])
            nc.vector.tensor_tensor(
                out=out_acc[:, blk * SUB + s: blk * SUB + s + 1],
                in0=o_r[:], in1=o_i[:], op=OP.add)

    nc.sync.dma_start(out[:], out_acc[:])


def _host_prep(h_i, t_i, r_i, d_i, emb_E_real, emb_E_img, emb_R_real,
               emb_R_img, time_table):
    """Host-side layout prep (table packing / index manipulation only)."""
    embEp = np.ascontiguousarray(
        np.concatenate([emb_E_real, emb_E_img], axis=1)).astype(
            ml_dtypes.bfloat16)                       # [N_ENTITY, 1024]
    embRp = np.ascontiguousarray(
        np.concatenate([emb_R_real, emb_R_img], axis=1)).astype(
            ml_dtypes.bfloat16)                       # [N_RELATION, 1024]

    tt = np.asarray(time_table, dtype=np.float32)
    cs = np.concatenate([np.cos(tt), np.sin(tt)], axis=1)  # [367, 1024]
    cs_pad = np.zeros((NI, DD), np.float32)
    cs_pad[0:4] = cs[PAD_DAY]          # prefix rows implement neg-idx clamp
    cs_pad[4:370] = cs[0:366]
    # forward table, chunked for SBUF [128, IC, DD]
    csF = np.ascontiguousarray(
        cs_pad.reshape(IC, P, DD).transpose(1, 0, 2).reshape(P, IC * DD)
    ).astype(ml_dtypes.bfloat16)
    # transposed table for SBUF [128, JC, NI]: csT[p, j, i] = cs_pad[i, j*128+p]
    csT = np.ascontiguousarray(
        cs_pad.T.reshape(JC, P, NI).transpose(1, 0, 2).reshape(P, JC * NI)
    ).astype(ml_dtypes.bfloat16)

    d = np.asarray(d_i, dtype=np.int64)
    i_grid = np.arange(NI, dtype=np.int64)[:, None]   # [NI, 1]

    ht = np.stack([h_i, t_i], axis=1).astype(np.int32)    # [B, 2]
    rx = np.asarray(r_i, dtype=np.int32).reshape(B, 1)

    def tileize(a):
        C = a.shape[1]
        return np.ascontiguousarray(
            a.reshape(T, P, C).transpose(1, 0, 2).reshape(P, T * C))

    in_maps = []
    perms = []
    fallback = []          # original global indices needing host fixup
    for core in range(N_CORES):
        sl = slice(core * BL, (core + 1) * BL)
        perm = np.argsort(d[sl], kind="stable")        # sorted-by-day order
        perms.append(perm)
        dl = d[sl][perm]                               # [BL] sorted
        # safety check: each sub-tile's window rows must fit its static
        # chunk set; collect violating elements for exact host fixup
        ds = dl.reshape(T, P)
        for s in range(T):
            lo, hi = int(ds[s].min()), int(ds[s].max()) + 4
            ok = np.zeros(NI, bool)
            for k in SB_SETS[s]:
                ok[k * P:(k + 1) * P] = True
            if not ok[lo:hi + 1].all():
                bad = np.arange(s * P, (s + 1) * P)
                fallback.extend(core * BL + perm[bad])
        m = ((i_grid >= dl[None, :]) & (i_grid <= dl[None, :] + 4))  # [NI, BL]
        maskT = np.ascontiguousarray(
            m.reshape(IC, P, BL).transpose(1, 0, 2).reshape(P, IC * BL)
        ).astype(ml_dtypes.bfloat16)
        in_maps.append(dict(
            embEp=embEp,
            embRp=embRp,
            csF=csF,
            csT=csT,
            maskT=maskT,
            ht_idx=tileize(ht[sl][perm]),
            r_idx=tileize(rx[sl][perm]),
        ))
    return in_maps, perms, np.asarray(fallback, dtype=np.int64)


def _reference_np(h_i, t_i, r_i, d_i, eR, eI, rR, rI, tt):
    """Exact numpy replica of the reference for rare host fixups."""
    n_day = tt.shape[0] - 2
    idx = d_i[:, None] - np.arange(W - 1, -1, -1)[None, :]
    idx = np.where(idx >= 0, idx, n_day)
    dl = tt[idx]
    s_, c_ = np.sin(dl), np.cos(dl)
    hr, hi = eR[h_i][:, None, :], eI[h_i][:, None, :]
    tr, ti = eR[t_i][:, None, :], eI[t_i][:, None, :]
    h_re, h_im = hr * c_ - hi * s_, hr * s_ + hi * c_
    t_re, t_im = tr * c_ - ti * s_, tr * s_ + ti * c_
    rr, ri = rR[r_i], rI[r_i]

    def soft(lg):
        e = np.exp(lg - lg.max(axis=1, keepdims=True))
        return (e / e.sum(axis=1, keepdims=True))[..., None]

    a_r = soft(np.einsum("bd,bwd->bw", rr, h_re))
    a_i = soft(np.einsum("bd,bwd->bw", ri, h_im))
    b_r = soft(np.einsum("bd,bwd->bw", rr, t_re))
    b_i = soft(np.einsum("bd,bwd->bw", ri, t_im))
    y_r = (a_r * h_re).sum(1)
    y_i = (a_i * h_im).sum(1)
    z_r = (b_r * t_re).sum(1)
    z_i = (b_i * t_im).sum(1)
    return (np.abs(y_r + rr - z_r).sum(1)
            + np.abs(y_i + ri + z_i).sum(1)).astype(np.float32)


def build_nc():
    nc = bacc.Bacc(
        "TRN2",
        target_bir_lowering=False,
        debug=False,
        enable_asserts=False,
        num_devices=N_CORES,
    )
    ins = dict(
        embEp=nc.dram_tensor("embEp", [N_ENTITY, DD], BF16,
                             kind="ExternalInput").ap(),
        embRp=nc.dram_tensor("embRp", [N_RELATION, DD], BF16,
                             kind="ExternalInput").ap(),
        csF=nc.dram_tensor("csF", [P, IC * DD], BF16,
                           kind="ExternalInput").ap(),
        csT=nc.dram_tensor("csT", [P, JC * NI], BF16,
                           kind="ExternalInput").ap(),
        maskT=nc.dram_tensor("maskT", [P, IC * BL], BF16,
                             kind="ExternalInput").ap(),
        ht_idx=nc.dram_tensor("ht_idx", [P, T * 2], I32,
                              kind="ExternalInput").ap(),
        r_idx=nc.dram_tensor("r_idx", [P, T], I32,
                             kind="ExternalInput").ap(),
    )
    outs = dict(
        out=nc.dram_tensor("out", [P, T], F32, kind="ExternalOutput").ap(),
    )
    with tile.TileContext(nc) as tc:
        _emit(tc, outs, ins)
    nc.compile()
    return nc


_NC_CACHE = {}


def kernel(h_i, t_i, r_i, d_i, emb_E_real, emb_E_img, emb_R_real, emb_R_img,
           time_table, _want_results=False, _trace=False):
    in_maps, perms, fallback = _host_prep(
        h_i, t_i, r_i, d_i, emb_E_real, emb_E_img, emb_R_real, emb_R_img,
        time_table)
    if "nc" not in _NC_CACHE:
        _NC_CACHE["nc"] = build_nc()
    nc = _NC_CACHE["nc"]
    res = run_bass_kernel_spmd(
        nc, in_maps, core_ids=list(range(N_CORES)), trace=_trace)
    out = np.empty((B,), np.float32)
    for core in range(N_CORES):
        o = np.asarray(res.results[core]["out"]).T.reshape(BL)  # sorted order
        out[core * BL + perms[core]] = o
    if len(fallback):
        f = np.asarray(fallback)
        out[f] = _reference_np(
            np.asarray(h_i)[f], np.asarray(t_i)[f], np.asarray(r_i)[f],
            np.asarray(d_i)[f], np.asarray(emb_E_real),
            np.asarray(emb_E_img), np.asarray(emb_R_real),
            np.asarray(emb_R_img), np.asarray(time_table, dtype=np.float32))
    if _want_results:
        return out, res
    return out
